# revision 27
# baseline (speedup 1.0000x reference)
"""Trainium2 Bass kernel: causal transformer encoder layer, SPMD on 8 cores.

v2: fp8(e4m3) DoubleRow matmuls for QKV / out-proj / FFN / AV (k-tile pairs
contract in one MM at 0.5 cyc/row), bf16 scores, PE-side additive causal
masks (fp8 identity-matmul accumulated into score PSUM), residual adds via
scaled-identity matmuls, DMA-transpose for the FFN input transpose, and
LayerNorm rstd = exp(-0.5*ln(var)) so every ACT function lives in one table
set (natural_log_exp_and_others).

Sharding: core c handles batch b = c//2; the two cores of a batch split the
2048 query positions into 4 chunks of 256, paired so both halves have the
same static structure E = [16,12,8,4] k-tiles; per-core differences live in
mask DATA (additive 0/-30 biases on scores over the last two k-tile pairs).

fp8 scaling (power-of-2, exact): wq' = (Wq^T/8)*512, wk' = Wk^T*64,
wv' = Wv^T*64, wo' = Wo^T*64, w1' = W1^T*64, w2' = W2^T*64.  Evacuations
rescale Q by 2^-9, K/V by 2^-6 back to true scale.  The ones-column of Vaug
is 2^-6 so ctx comes out 64x true; out-proj PSUM is then 4096x true and the
residual enters via identity*4096 (bf16).  FFN: relu evac scales 2^-4 so hb
is 4x true, FFN2 PSUM is 256x true, residual via identity*256.  LayerNorm is
scale-invariant, so both LN outputs are true-scale.
"""

from contextlib import ExitStack
from dataclasses import dataclass

import numpy as np
import ml_dtypes

import concourse.bass as bass
import concourse.bacc as bacc
import concourse.tile as tile
from concourse import mybir
from concourse.bass_utils import run_bass_kernel_spmd
from concourse.masks import make_identity

F32 = mybir.dt.float32
F16 = mybir.dt.float16
BF16 = mybir.dt.bfloat16
F8 = mybir.dt.float8e4
AF = mybir.ActivationFunctionType
ALU = mybir.AluOpType

EPS = 1e-5
NEG = -30.0  # additive mask value (exp(-30) ~ 1e-13); exact in fp8e4m3
VPAD = 80    # per-head stride in Vaug (64 V + 1 ones + 15 pad, 16B aligned)


@dataclass
class Cfg:
    B: int = 4
    S: int = 2048
    D: int = 512
    F: int = 2048
    H: int = 8
    CHUNK: int = 256
    KT: int = 128
    use_bq: bool = False
    use_bk: bool = False
    use_bv: bool = False
    use_b1: bool = False
    use_b2: bool = False
    use_g1: bool = False
    use_bn1: bool = False
    use_g2: bool = False
    use_bn2: bool = False

    @property
    def HD(self):
        return self.D // self.H

    @property
    def DK(self):
        return self.D // 128

    @property
    def FK(self):
        return self.F // 128

    @property
    def NCH(self):
        return self.S // self.CHUNK

    @property
    def NQ(self):
        return (self.NCH // 2) * self.CHUNK

    @property
    def NSLOT(self):
        return self.NCH // 2

    @property
    def QT(self):
        return self.NQ // 128

    def ext(self, ci):
        return ((ci + 1) * self.CHUNK) // self.KT

    def slot_chunks(self, half):
        n = self.NCH
        if half == 0:
            s = [i for i in range(n) if i % 4 in (0, 3)]
        else:
            s = [i for i in range(n) if i % 4 in (1, 2)]
        return sorted(s, key=lambda ci: -self.ext(ci))

    def slot_qs(self, half):
        return [ci * self.CHUNK for ci in self.slot_chunks(half)]

    def slot_E(self):
        a = self.slot_chunks(0)
        b = self.slot_chunks(1)
        E = [max(self.ext(x), self.ext(y)) for x, y in zip(a, b)]
        for e in E:
            assert e % 4 == 0, E
        return E


def build_nc(cfg: Cfg, n_bodies: int = 1) -> bass.Bass:
    S, D, F, H, HD = cfg.S, cfg.D, cfg.F, cfg.H, cfg.HD
    DK, FK, QT, NQ, CHUNK, KT = cfg.DK, cfg.FK, cfg.QT, cfg.NQ, cfg.CHUNK, cfg.KT
    NSLOT = cfg.NSLOT
    E = cfg.slot_E()
    WQ = min(512, NQ)
    WS = min(512, S)

    nc = bacc.Bacc("TRN2", target_bir_lowering=False)

    xT_d = nc.declare_dram_parameter("xT", [D, S], F8, isOutput=False)
    xqT_d = nc.declare_dram_parameter("xqT", [D, NQ], F8, isOutput=False)
    # residual x for own tokens arrives as the fp8 QUANTIZATION ERROR of xqT
    # (plus bias folds); on-device PE transposes of xqT + this error term
    # reconstruct bf16-precision token-major x at half the staged bytes.
    xoerr_d = nc.declare_dram_parameter("xoerr", [NQ, D], F8, isOutput=False)
    # weights are static across calls: embed as NEFF Const tensors (DMA'd to
    # HBM once at model load) instead of per-call staged inputs.  _WEIGHTS is
    # set by kernel() before build; the nc cache is keyed on its content.
    w = _WEIGHTS
    wqkv_d = nc.inline_tensor(w["wqkvT"], "wqkvT")
    wo_d = nc.inline_tensor(w["woT"], "woT")
    w1_d = nc.inline_tensor(w["w1T"], "w1T")
    w2_d = nc.inline_tensor(w["w2T"], "w2T")
    # only-if-used parameter declarations: absent params are absent inputs,
    # and host->device staging of the inputs dominates the e2e time.
    bq_d = nc.declare_dram_parameter("bq", [D], F32, isOutput=False) if cfg.use_bq else None
    bk_d = nc.declare_dram_parameter("bk", [D], F32, isOutput=False) if cfg.use_bk else None
    bv_d = nc.declare_dram_parameter("bv", [D], F32, isOutput=False) if cfg.use_bv else None
    b1_d = nc.declare_dram_parameter("b1", [F], F32, isOutput=False) if cfg.use_b1 else None
    b2_d = nc.declare_dram_parameter("b2", [128, D], F32, isOutput=False) if cfg.use_b2 else None
    g1_d = nc.declare_dram_parameter("g1v", [128, D], F32, isOutput=False) if cfg.use_g1 else None
    bn1_d = nc.declare_dram_parameter("bn1v", [128, D], F32, isOutput=False) if cfg.use_bn1 else None
    g2_d = nc.declare_dram_parameter("g2v", [128, D], F32, isOutput=False) if cfg.use_g2 else None
    bn2_d = nc.declare_dram_parameter("bn2v", [128, D], F32, isOutput=False) if cfg.use_bn2 else None
    # additive score masks for the last two k-tile pairs of each slot:
    # [128 keys, slot, pair-in-{T-2,T-1}, jj x 256 queries].  Content is
    # head-independent, so both hh matmuls read the same 512-col slice.
    masks_d = nc.declare_dram_parameter(
        "masks", [128, NSLOT, 2, 2 * CHUNK], F8, isOutput=False
    )
    out_d = nc.declare_dram_parameter("out", [NQ, D], F16, isOutput=True)

    with ExitStack() as top:
        tc = top.enter_context(tile.TileContext(nc, pool_alloc_mode="queue"))
        consts = top.enter_context(tc.tile_pool(name="consts", bufs=1))

        ident_f8 = consts.tile([128, 128], F8)
        make_identity(nc, ident_f8)
        ident128_f8 = consts.tile([128, 128], F8)
        nc.gpsimd.memset(ident128_f8, 0.0)
        nc.gpsimd.affine_select(
            out=ident128_f8, in_=ident128_f8, compare_op=ALU.not_equal,
            fill=128.0, base=0, pattern=[[-1, 128]], channel_multiplier=1,
        )
        ident256 = consts.tile([128, 128], BF16)
        nc.gpsimd.memset(ident256, 0.0)
        nc.gpsimd.affine_select(
            out=ident256, in_=ident256, compare_op=ALU.not_equal,
            fill=256.0, base=0, pattern=[[-1, 128]], channel_multiplier=1,
        )

        masks_sb = consts.tile([128, NSLOT, 2, 2 * CHUNK], F8)

        eps_sb = consts.tile([128, 1], F32)
        nc.vector.memset(eps_sb, EPS)

        bq_sb = bk_sb = bv_sb = b1_sb = None
        if cfg.use_bq:
            bq_sb = consts.tile([128, DK], F32)
            nc.sync.dma_start(out=bq_sb, in_=bq_d.rearrange("(m p) -> p m", p=128))
        if cfg.use_bk:
            bk_sb = consts.tile([128, DK], F32)
            nc.sync.dma_start(out=bk_sb, in_=bk_d.rearrange("(m p) -> p m", p=128))
        if cfg.use_bv:
            bv_sb = consts.tile([128, DK], F32)
            nc.sync.dma_start(out=bv_sb, in_=bv_d.rearrange("(m p) -> p m", p=128))
        if cfg.use_b1:
            b1_sb = consts.tile([128, FK], F32)
            nc.sync.dma_start(out=b1_sb, in_=b1_d.rearrange("(f p) -> p f", p=128))

        def bcast_const(dram, nm):
            t = consts.tile([128, D], F32, name=nm, tag=nm)
            nc.sync.dma_start(out=t, in_=dram[:, :])
            return t

        g1_b = bcast_const(g1_d, "g1b") if cfg.use_g1 else None
        bn1_b = bcast_const(bn1_d, "bn1b") if cfg.use_bn1 else None
        g2_b = bcast_const(g2_d, "g2b") if cfg.use_g2 else None
        bn2_b = bcast_const(bn2_d, "bn2b") if cfg.use_bn2 else None
        b2_b = bcast_const(b2_d, "b2b") if cfg.use_b2 else None

        def emit_body():
            pctx_cm = tc.tile_pool(name="pctx", bufs=1)
            pctx = pctx_cm.__enter__()
            # per-slot ctx tiles so out-proj can start as each slot finishes;
            # ctx_b is the bf16 token-major->feature-major transpose staging
            ctx_s = [pctx.tile([128, DK, CHUNK], F8, name=f"ctx{s}", tag=f"ctx{s}")
                     for s in range(NSLOT)]
            ctx_b = [pctx.tile([128, DK, CHUNK], BF16, name=f"ctb{s}", tag=f"ctb{s}")
                     for s in range(NSLOT)]

            pqkv_cm = tc.tile_pool(name="pqkv", bufs=1)
            pqkv = pqkv_cm.__enter__()

            # ---------------- Phase A: QKV projections (fp8 DoubleRow) -------
            pa_cm = tc.tile_pool(name="pa", bufs=1)
            pa = pa_cm.__enter__()

            NKP = DK // 2
            xT_sb = [pa.tile([128, 2, S], F8, name=f"xT{p}", tag=f"xT{p}")
                     for p in range(NKP)]
            # xqT lives in the long-lived pctx pool: phase C re-reads it to
            # reconstruct the bf16 residual x from fp8 + error term.
            xqT_sb = [pctx.tile([128, 2, NQ], F8, name=f"xq{p}", tag=f"xq{p}")
                      for p in range(NKP)]
            wqkv_sb = [pa.tile([128, 2, 3 * D], F8, name=f"wq{p}", tag=f"wq{p}")
                       for p in range(NKP)]
            xT_r = xT_d.rearrange("(m p) t -> p m t", p=128)
            xqT_r = xqT_d.rearrange("(m p) t -> p m t", p=128)
            wqkv_r = wqkv_d.rearrange("(m p) c -> p m c", p=128)
            for p in range(NKP):
                # issue on both HWDGE queues so the first Q inputs land
                # without serializing behind each other on SP
                nc.scalar.dma_start(out=xqT_sb[p], in_=xqT_r[:, 2 * p : 2 * p + 2, :])
                nc.sync.dma_start(out=wqkv_sb[p], in_=wqkv_r[:, 2 * p : 2 * p + 2, :])
            for p in range(NKP):
                nc.sync.dma_start(out=xT_sb[p], in_=xT_r[:, 2 * p : 2 * p + 2, :])
            nc.scalar.dma_start(out=masks_sb, in_=masks_d[:, :, :, :])

            Qfm = [pqkv.tile([128, NQ], BF16, name=f"Qfm{m}", tag=f"Qfm{m}")
                   for m in range(DK)]
            Kfm = [pqkv.tile([128, S], BF16, name=f"Kfm{m}", tag=f"Kfm{m}")
                   for m in range(DK)]
            NVG = (S // KT) // 4
            Vaug = [pqkv.tile([128, 4, H * VPAD], F8, name=f"Va{g}", tag=f"Va{g}")
                    for g in range(NVG)]
            # ones columns (2^-6 so ctx comes out 64x true)
            for g in range(NVG):
                nc.vector.memset(
                    Vaug[g].rearrange("p t (h c) -> p t h c", h=H)[:, :, :, HD : HD + 1],
                    0.015625,
                )

            with tc.tile_pool(name="pa_psum", bufs=7, space="PSUM") as pap:
                def emit_q(m):
                    for ch in range(NQ // WQ):
                        ps = pap.tile([128, WQ], F32, tag="ps")
                        for kp in range(NKP):
                            nc.tensor.matmul(
                                out=ps,
                                lhsT=wqkv_sb[kp][:, :, m * 128 : (m + 1) * 128],
                                rhs=xqT_sb[kp][:, :, ch * WQ : (ch + 1) * WQ],
                                start=(kp == 0),
                                stop=(kp == NKP - 1),
                                perf_mode=mybir.MatmulPerfMode.DoubleRow,
                            )
                        dst = Qfm[m][:, ch * WQ : (ch + 1) * WQ]
                        nc.scalar.activation(
                            out=dst, in_=ps, func=AF.Identity,
                            bias=(bq_sb[:, m : m + 1] if cfg.use_bq else 0.0),
                            scale=1.0 / 512.0,
                        )

                def emit_k(ch):
                    for m in range(DK):
                        ps = pap.tile([128, WS], F32, tag="ps")
                        for kp in range(NKP):
                            nc.tensor.matmul(
                                out=ps,
                                lhsT=wqkv_sb[kp][:, :, D + m * 128 : D + (m + 1) * 128],
                                rhs=xT_sb[kp][:, :, ch * WS : (ch + 1) * WS],
                                start=(kp == 0),
                                stop=(kp == NKP - 1),
                                perf_mode=mybir.MatmulPerfMode.DoubleRow,
                            )
                        dst = Kfm[m][:, ch * WS : (ch + 1) * WS]
                        if cfg.use_bk:
                            nc.vector.tensor_scalar(
                                out=dst, in0=ps, scalar1=1.0 / 64.0,
                                scalar2=bk_sb[:, m : m + 1],
                                op0=ALU.mult, op1=ALU.add,
                            )
                        elif (ch * DK + m) % 2 == 0:
                            nc.scalar.activation(
                                out=dst, in_=ps, func=AF.Identity, scale=1.0 / 64.0
                            )
                        else:
                            nc.vector.tensor_scalar_mul(dst, ps, 1.0 / 64.0)

                def emit_v(g):
                    for t in range(4 * g, 4 * g + 4):
                        ps = pap.tile([128, D], F32, tag="ps")
                        for kp in range(NKP):
                            nc.tensor.matmul(
                                out=ps,
                                lhsT=xT_sb[kp][:, :, t * 128 : (t + 1) * 128],
                                rhs=wqkv_sb[kp][:, :, 2 * D : 3 * D],
                                start=(kp == 0),
                                stop=(kp == NKP - 1),
                                perf_mode=mybir.MatmulPerfMode.DoubleRow,
                            )
                        vdst = Vaug[t // 4][:, t % 4, :].rearrange("p (h c) -> p h c", h=H)
                        src = ps.rearrange("p (h c) -> p h c", h=H)
                        if t % 2 == 0:
                            nc.scalar.activation(
                                out=vdst[:, :, 0:HD], in_=src, func=AF.Identity,
                                scale=1.0 / 64.0,
                            )
                        else:
                            nc.vector.tensor_scalar_mul(
                                vdst[:, :, 0:HD], src, 1.0 / 64.0
                            )

                # interleave by attention consumption order (small slots first)
                emit_q(0)
                emit_k(0)
                emit_v(0)
                emit_q(1)
                emit_k(1)
                emit_v(1)
                emit_q(2)
                emit_q(3)
                emit_k(2)
                emit_v(2)
                emit_k(3)
                emit_v(3)

            pa_cm.__exit__(None, None, None)

            # ---------------- Phase B: attention -----------------------------
            with (
                tc.tile_pool(name="pb_sc", bufs=3, space="PSUM") as pbs,
                tc.tile_pool(name="pb_cx", bufs=2, space="PSUM") as pbc,
                tc.tile_pool(name="pb_es", bufs=5) as pbe,
                tc.tile_pool(name="pb_w", bufs=4) as pbw,
            ):
                sch_ctr = 0  # round-robin unmasked exp tiles between ACT/DVE
                slot_order = sorted(range(NSLOT), key=lambda s: E[s])
                for s in slot_order:
                    T = E[s] // 2
                    for m in range(DK):  # head pair (2m, 2m+1)
                        cps = pbc.tile([128, 2, 2, HD + 1], F32, tag="cps")
                        for t in range(T):
                            masked = t >= T - 2
                            sc = pbs.tile([128, 4 * CHUNK], F32, tag="sc")
                            for hh in range(2):
                                # each hh occupies one PSUM bank of sc; the
                                # FIRST matmul into the bank uses start=True
                                # (bank-wide has_written clear), all later
                                # ones start=False.
                                off = hh * HD
                                for jj in range(2):
                                    j = 2 * t + jj
                                    nc.tensor.matmul(
                                        out=sc[:, (2 * hh + jj) * CHUNK : (2 * hh + jj + 1) * CHUNK],
                                        lhsT=Kfm[m][off : off + HD, j * KT : (j + 1) * KT],
                                        rhs=Qfm[m][off : off + HD, s * CHUNK : (s + 1) * CHUNK],
                                        start=(jj == 0),
                                        stop=(jj == 1 and not masked),
                                        skip_group_check=True,
                                    )
                                if masked:
                                    # additive causal/equalization mask
                                    nc.tensor.matmul(
                                        out=sc[:, 2 * hh * CHUNK : (2 * hh + 2) * CHUNK],
                                        lhsT=ident_f8,
                                        rhs=masks_sb[:, s, t - (T - 2), :],
                                        start=False,
                                        stop=True,
                                        skip_group_check=True,
                                    )
                            es = pbe.tile([128, 4 * CHUNK], F8, tag="es")
                            # fp8 Schraudolph exp on DVE for ~29/80 tiles (the
                            # e4m3 bit pattern of exp(x) is ~ int8(8/ln2*x+56));
                            # unmasked scores are within +-1.5, and -30-masked
                            # scores saturate the int8 convert to -128 = -0.0
                            # in e4m3 (HW-verified), so every tile is eligible.
                            # Bresenham spread keeps ACT/DVE interleaved.
                            if (sch_ctr * 29) % 80 < 29:
                                nc.vector.tensor_scalar(
                                    out=es.bitcast(mybir.dt.int8),
                                    in0=sc,
                                    scalar1=11.5416,
                                    scalar2=56.0,
                                    op0=ALU.mult,
                                    op1=ALU.add,
                                )
                            else:
                                nc.scalar.activation(out=es, in_=sc, func=AF.Exp)
                            sch_ctr += 1
                            for hh in range(2):
                                # transposed AV: queries on partitions, 65 V
                                # columns (incl. 2^-6 ones) in the free dim —
                                # all four (qh, hh) streams accumulate in ONE
                                # PSUM bank; start=True only on the first MM.
                                h = 2 * m + hh
                                g, tg = (2 * t) // 4, (2 * t) % 4
                                es_hh = es[:, 2 * hh * CHUNK : 2 * (hh + 1) * CHUNK].rearrange(
                                    "p (two q) -> p two q", two=2
                                )
                                for qh in range(2):
                                    nc.tensor.matmul(
                                        out=cps[:, qh, hh, :],
                                        lhsT=es_hh[:, :, qh * 128 : (qh + 1) * 128],
                                        rhs=Vaug[g][:, tg : tg + 2, h * VPAD : h * VPAD + HD + 1],
                                        start=(t == 0 and hh == 0 and qh == 0),
                                        stop=(t == T - 1 and hh == 1 and qh == 1),
                                        perf_mode=mybir.MatmulPerfMode.DoubleRow,
                                        skip_group_check=True,
                                    )
                        inv = pbw.tile([128, 2, 2], F32, tag="inv")
                        for qh in range(2):
                            nc.vector.reciprocal(
                                out=inv[:, qh, :], in_=cps[:, qh, :, HD : HD + 1]
                            )
                        ctm = pbw.tile([128, 2, 128], BF16, tag="ctm")
                        for qh in range(2):
                            for hh in range(2):
                                nc.vector.tensor_scalar_mul(
                                    ctm[:, qh, hh * HD : (hh + 1) * HD],
                                    cps[:, qh, hh, 0:HD],
                                    inv[:, qh, hh : hh + 1],
                                )
                        for qh in range(2):
                            nc.sync.dma_start_transpose(
                                out=ctx_b[s][:, m, qh * 128 : (qh + 1) * 128],
                                in_=ctm[:, qh, :],
                            )
                    # all head-pairs transposed: widen to fp8 for out-proj
                    nc.gpsimd.tensor_copy(ctx_s[s], ctx_b[s])

            pqkv_cm.__exit__(None, None, None)

            # -------- late-weight loads (overlap attention) -------------------
            pc_cm = tc.tile_pool(name="pc", bufs=1)
            pc = pc_cm.__enter__()
            wo_sb = pc.tile([128, DK, D], F8)
            nc.sync.dma_start(out=wo_sb, in_=wo_d.rearrange("(m p) c -> p m c", p=128))
            xoerr_sb = pc.tile([128, QT, D], F8)
            nc.sync.dma_start(out=xoerr_sb, in_=xoerr_d.rearrange("(t p) d -> p t d", p=128))
            pw1_cm = tc.tile_pool(name="pw1", bufs=1)
            pw1 = pw1_cm.__enter__()
            w1_sb = pw1.tile([128, DK, F], F8)
            nc.sync.dma_start(out=w1_sb, in_=w1_d.rearrange("(m p) c -> p m c", p=128))

            # ---------------- Phase C: out-proj + LN1 + transpose -------------
            # token-tile halves so FFN chunk 1 can start mid-phase-C
            pd_cm = tc.tile_pool(name="pd", bufs=1)
            pd = pd_cm.__enter__()
            NCHD = NQ // WQ
            NSUB = WQ // 128
            xln1 = [pd.tile([128, NSUB, D], BF16, name=f"xl{c}", tag=f"xl{c}")
                    for c in range(NCHD)]
            x1tb = [pd.tile([128, DK, WQ], BF16, name=f"xb{c}", tag=f"xb{c}")
                    for c in range(NCHD)]
            x1t = [pd.tile([128, DK, WQ], F8, name=f"xf{c}", tag=f"xf{c}")
                   for c in range(NCHD)]

            pw2_cm = tc.tile_pool(name="pw2", bufs=1)
            pw2 = pw2_cm.__enter__()
            w2_sb = pw2.tile([128, FK, D], F8)
            nc.sync.dma_start(out=w2_sb, in_=w2_d.rearrange("(f p) c -> p f c", p=128))

            def layer_norm_step(tt, g_b, bn_b, dst, work, act_stats=False):
                # tt: [128, D] fp32 (PSUM ok); scale-invariant LN
                if act_stats:
                    # stats via ACT accum_out (sum / sum-of-squares reduced
                    # during the copy passes) — keeps DVE free in the tail
                    xraw = work.tile([128, D], F32, tag="xraw")
                    ssum = work.tile([128, 1], F32, tag="ssum")
                    nc.scalar.activation(
                        out=xraw, in_=tt, func=AF.Identity, accum_out=ssum
                    )
                    junk = work.tile([128, D], F32, tag="junk")
                    ssq = work.tile([128, 1], F32, tag="ssq")
                    nc.scalar.activation(
                        out=junk, in_=tt, func=AF.Square, accum_out=ssq
                    )
                    mu = work.tile([128, 1], F32, tag="mu")
                    nc.vector.tensor_scalar_mul(mu, ssum, 1.0 / D)
                    musq = work.tile([128, 1], F32, tag="musq")
                    nc.vector.tensor_mul(musq, mu, mu)
                    var = work.tile([128, 1], F32, tag="var")
                    nc.vector.tensor_scalar(
                        out=var, in0=ssq, scalar1=1.0 / D, scalar2=musq,
                        op0=ALU.mult, op1=ALU.subtract,
                    )
                    sd = work.tile([128, 1], F32, tag="sd")
                    nc.scalar.activation(out=sd, in_=var, func=AF.Sqrt, bias=eps_sb)
                    rstd = work.tile([128, 1], F32, tag="rstd")
                    nc.vector.reciprocal(out=rstd, in_=sd)
                    nc.vector.tensor_scalar(
                        out=dst, in0=xraw, scalar1=mu, scalar2=rstd,
                        op0=ALU.subtract, op1=ALU.mult,
                    )
                else:
                    stats = work.tile([128, nc.vector.BN_STATS_DIM], F32, tag="stats")
                    nc.vector.bn_stats(out=stats, in_=tt)
                    mv = work.tile([128, nc.vector.BN_AGGR_DIM], F32, tag="mv")
                    nc.vector.bn_aggr(out=mv, in_=stats)
                    sd = work.tile([128, 1], F32, tag="sd")
                    nc.scalar.activation(out=sd, in_=mv[:, 1:2], func=AF.Sqrt, bias=eps_sb)
                    rstd = work.tile([128, 1], F32, tag="rstd")
                    nc.vector.reciprocal(out=rstd, in_=sd)
                    nc.vector.tensor_scalar(
                        out=dst, in0=tt, scalar1=mv[:, 0:1], scalar2=rstd,
                        op0=ALU.subtract, op1=ALU.mult,
                    )
                if g_b is not None:
                    nc.vector.tensor_mul(dst, dst, g_b)
                if bn_b is not None:
                    nc.vector.tensor_add(dst, dst, bn_b)

            # process token tiles in the order their slots finish attention
            t_order = [2 * s + i for s in slot_order for i in range(2)]
            ch_order = []
            for t in t_order:
                if t // NSUB not in ch_order:
                    ch_order.append(t // NSUB)
            with (
                tc.tile_pool(name="pc_ps", bufs=4, space="PSUM") as pcp,
                tc.tile_pool(name="pc_w", bufs=3) as pcw,
            ):
                done = set()
                conv_done = set()
                for t in t_order:
                    ch, sub = t // NSUB, t % NSUB
                    ps = pcp.tile([128, D], F32, tag="ps")
                    for kp in range(DK // 2):
                        nc.tensor.matmul(
                            out=ps,
                            lhsT=ctx_s[t // 2][:, 2 * kp : 2 * kp + 2,
                                               (t % 2) * 128 : (t % 2 + 1) * 128],
                            rhs=wo_sb[:, 2 * kp : 2 * kp + 2, :],
                            start=(kp == 0),
                            stop=False,
                            perf_mode=mybir.MatmulPerfMode.DoubleRow,
                        )
                    # residual: out-proj PSUM is 128x true (wo' = 2*Wo^T).
                    # fp8 part: transpose-inject xqT blocks against 128*I;
                    # error part: token-major fp8 xoerr against 128*I.
                    # (128 is the largest power of 2 in f8e4's range, max 240.)
                    for m in range(DK):
                        nc.tensor.matmul(
                            out=ps[:, m * 128 : (m + 1) * 128],
                            lhsT=xqT_sb[m // 2][:, m % 2, t * 128 : (t + 1) * 128],
                            rhs=ident128_f8,
                            start=False,
                            stop=False,
                            skip_group_check=True,
                        )
                    nc.tensor.matmul(
                        out=ps,
                        lhsT=ident128_f8,
                        rhs=xoerr_sb[:, t, :],
                        start=False,
                        stop=True,
                        skip_group_check=True,
                    )
                    layer_norm_step(ps, g1_b, bn1_b, xln1[ch][:, sub, :], pcw)
                    for mm in range(DK):
                        nc.sync.dma_start_transpose(
                            out=x1tb[ch][:, mm, sub * 128 : (sub + 1) * 128],
                            in_=xln1[ch][:, sub, mm * 128 : (mm + 1) * 128],
                        )
                    done.add(t)
                    for c2 in range(NCHD):
                        if c2 not in conv_done and all(
                            c2 * NSUB + i in done for i in range(NSUB)
                        ):
                            conv_done.add(c2)
                            for mm in range(DK):
                                # bf16 -> fp8 on the otherwise-idle gpsimd
                                nc.gpsimd.tensor_copy(
                                    x1t[c2][:, mm, :], x1tb[c2][:, mm, :]
                                )

            # ---------------- Phase D: FFN + LN2 + store ----------------------
            # hb is split into per-f-pair tiles so FFN2 trails FFN1 at
            # f-granularity; FFN2 runs in sub-pairs with double-buffered yps
            # so the next chunk isn't blocked on LN2 draining.
            with (
                tc.tile_pool(name="pf_h", bufs=4, space="PSUM") as pfh,
                tc.tile_pool(name="pf_y", bufs=2, space="PSUM") as pfy,
                tc.tile_pool(name="pf_hb", bufs=3) as pfhb,
                tc.tile_pool(name="pf_w", bufs=3) as pfw,
                tc.tile_pool(name="pf_o", bufs=3) as pfo,
            ):
                for ch in ch_order:
                    hb_p = [pfhb.tile([128, 2, WQ], F8, name=f"hb{i}", tag=f"hb{i}")
                            for i in range(FK // 2)]
                    for f in range(FK):
                        hp = pfh.tile([128, WQ], F32, tag="hp")
                        for kp in range(DK // 2):
                            nc.tensor.matmul(
                                out=hp,
                                lhsT=w1_sb[:, 2 * kp : 2 * kp + 2, f * 128 : (f + 1) * 128],
                                rhs=x1t[ch][:, 2 * kp : 2 * kp + 2, :],
                                start=(kp == 0),
                                stop=(kp == DK // 2 - 1),
                                perf_mode=mybir.MatmulPerfMode.DoubleRow,
                            )
                        # hb = 4x true = relu(psum * 2^-4).  The last chunk's
                        # relus run in the tail where DVE is free: split 50/50
                        # there; earlier chunks keep ACT-light 1-in-4 on DVE.
                        hdst = hb_p[f // 2][:, f % 2, :]
                        dve_relu = f % 4 == 3
                        if cfg.use_b1:
                            nc.scalar.activation(
                                out=hdst, in_=hp, func=AF.Relu,
                                bias=b1_sb[:, f : f + 1], scale=1.0 / 16.0,
                            )
                        elif dve_relu:
                            nc.vector.tensor_scalar(
                                out=hdst, in0=hp,
                                scalar1=1.0 / 16.0, scalar2=0.0,
                                op0=ALU.mult, op1=ALU.max,
                            )
                        else:
                            nc.scalar.activation(
                                out=hdst, in_=hp, func=AF.Relu,
                                scale=1.0 / 16.0,
                            )
                    for half in range(NSUB // 2):
                        yps = [pfy.tile([128, D], F32, name=f"y{i}", tag=f"y{i}")
                               for i in range(2)]
                        for fp in range(FK // 2):
                            for s2 in range(2):
                                c0 = (2 * half + s2) * 128
                                nc.tensor.matmul(
                                    out=yps[s2],
                                    lhsT=hb_p[fp][:, :, c0 : c0 + 128],
                                    rhs=w2_sb[:, 2 * fp : 2 * fp + 2, :],
                                    start=(fp == 0),
                                    stop=False,
                                    perf_mode=mybir.MatmulPerfMode.DoubleRow,
                                )
                        for s2 in range(2):
                            sub = 2 * half + s2
                            t = ch * NSUB + sub
                            # residual: += 256 * xln1 (FFN2 PSUM is 256x true)
                            nc.tensor.matmul(
                                out=yps[s2],
                                lhsT=ident256,
                                rhs=xln1[ch][:, sub, :],
                                start=False,
                                stop=True,
                                skip_group_check=True,
                            )
                            ob = pfo.tile([128, D], F16, tag="ob")
                            # final-chunk LN2s: run one of each pair on ACT so
                            # the two chains go in parallel in the drain tail
                            layer_norm_step(
                                yps[s2], g2_b, bn2_b, ob, pfw,
                                act_stats=(ch == ch_order[-1] and s2 == 1),
                            )
                            if cfg.use_b2:
                                nc.vector.tensor_add(ob, ob, b2_b)
                            nc.sync.dma_start(
                                out=out_d[t * 128 : (t + 1) * 128, :], in_=ob
                            )

            pw2_cm.__exit__(None, None, None)
            pd_cm.__exit__(None, None, None)
            pw1_cm.__exit__(None, None, None)
            pc_cm.__exit__(None, None, None)
            pctx_cm.__exit__(None, None, None)

        for _ in range(n_bodies):
            emit_body()

    nc.compile()
    return nc


# ---------------------------------------------------------------------------
# host side
# ---------------------------------------------------------------------------

def build_masks(cfg: Cfg, half: int) -> np.ndarray:
    """Additive masks [128, NSLOT, 2, 2*CHUNK] fp8: 0 keep / NEG drop.

    Layout of last dim: (jj 2) x CHUNK queries (head-independent); tile
    index dim2 selects k-tile pair T-2 or T-1 of the slot.
    """
    E = cfg.slot_E()
    chunks = cfg.slot_chunks(half)
    qs_l = cfg.slot_qs(half)
    m = np.zeros((128, cfg.NSLOT, 2, 2 * cfg.CHUNK), np.float32)
    k_loc = np.arange(128)[:, None]
    q_loc = np.arange(cfg.CHUNK)[None, :]
    for s, (ci, qs) in enumerate(zip(chunks, qs_l)):
        T = E[s] // 2
        for ti, tpair in enumerate((T - 2, T - 1)):
            for jj in range(2):
                j = 2 * tpair + jj
                keep = (qs + q_loc) >= (j * cfg.KT + k_loc)
                bias = np.where(keep, 0.0, NEG)
                m[:, s, ti, jj * cfg.CHUNK : (jj + 1) * cfg.CHUNK] = bias
    return m.astype(ml_dtypes.float8_e4m3)


def prepare_weights(inputs: dict, cfg: Cfg) -> dict:
    """fp8-scaled transposed weight tensors baked into the NEFF as constants."""
    Wqkv = np.asarray(inputs["Wqkv"], np.float32)
    Wo = np.asarray(inputs["Wo"], np.float32)
    W1 = np.asarray(inputs["W1"], np.float32)
    W2 = np.asarray(inputs["W2"], np.float32)
    D = cfg.D
    scale = 1.0 / np.sqrt(np.float32(cfg.HD))
    f8 = ml_dtypes.float8_e4m3
    wqkvT = np.concatenate(
        [
            np.ascontiguousarray(Wqkv[0:D].T) * (scale * 512.0),
            np.ascontiguousarray(Wqkv[D : 2 * D].T) * 64.0,
            np.ascontiguousarray(Wqkv[2 * D : 3 * D].T) * 64.0,
        ],
        axis=1,
    )
    return {
        # wo' at 2x (not 64x) so the out-proj PSUM sits at 128x true scale,
        # letting the residual enter via fp8 128*I identity matmuls.
        "wqkvT": wqkvT.astype(f8),
        "woT": (np.ascontiguousarray(Wo.T) * 2.0).astype(f8),
        "w1T": (np.ascontiguousarray(W1.T) * 64.0).astype(f8),
        "w2T": (np.ascontiguousarray(W2.T) * 64.0).astype(f8),
    }


def host_prepare(inputs: dict, cfg: Cfg):
    x = np.asarray(inputs["x"], np.float32)
    bqkv = np.asarray(inputs["bqkv"], np.float32)
    Wo = np.asarray(inputs["Wo"], np.float32)
    bo = np.asarray(inputs["bo"], np.float32)
    b1 = np.asarray(inputs["b1"], np.float32)
    b2 = np.asarray(inputs["b2"], np.float32)
    g1 = np.asarray(inputs["g1"], np.float32)
    bn1 = np.asarray(inputs["bn1"], np.float32)
    g2 = np.asarray(inputs["g2"], np.float32)
    bn2 = np.asarray(inputs["bn2"], np.float32)

    D = cfg.D
    scale = 1.0 / np.sqrt(np.float32(cfg.HD))
    f8 = ml_dtypes.float8_e4m3
    bq = bqkv[0:D] * scale
    bk = bqkv[D : 2 * D]
    bv = bqkv[2 * D : 3 * D]

    masks = [build_masks(cfg, half) for half in (0, 1)]

    in_maps = []
    own_idx_per_core = []
    for c in range(2 * cfg.B):
        b = c // 2
        half = c % 2
        own_idx = np.concatenate(
            [np.arange(qs, qs + cfg.CHUNK) for qs in cfg.slot_qs(half)]
        )
        own_idx_per_core.append(own_idx)
        xb = x[b]
        xown = xb[own_idx] + bo[None, :]
        if cfg.use_bv:
            # bv flows through attention as a constant added to every v;
            # since attention rows sum to 1, it adds bv @ Wo^T to attn_out.
            xown = xown + (bv @ Wo.T)[None, :]
        xq8 = xb[own_idx].astype(f8)
        im = {
            "xT": np.ascontiguousarray(xb.T).astype(f8),
            "xqT": np.ascontiguousarray(xq8.T),
            "xoerr": (xown - xq8.astype(np.float32)).astype(f8),
            "masks": masks[half],
        }
        if cfg.use_bq:
            im["bq"] = bq
        if cfg.use_bk:
            im["bk"] = bk
        if cfg.use_bv:
            im["bv"] = bv
        if cfg.use_b1:
            im["b1"] = b1 * 4.0
        if cfg.use_b2:
            im["b2"] = np.tile(b2[None, :], (128, 1))
        if cfg.use_g1:
            im["g1v"] = np.tile(g1[None, :], (128, 1))
        if cfg.use_bn1:
            im["bn1v"] = np.tile(bn1[None, :], (128, 1))
        if cfg.use_g2:
            im["g2v"] = np.tile(g2[None, :], (128, 1))
        if cfg.use_bn2:
            im["bn2v"] = np.tile(bn2[None, :], (128, 1))
        in_maps.append(im)
    return in_maps, own_idx_per_core


def make_cfg(inputs: dict) -> Cfg:
    x = np.asarray(inputs["x"])
    B, S, D = x.shape
    F = np.asarray(inputs["W1"]).shape[0]
    bqkv = np.asarray(inputs["bqkv"], np.float32)
    cfg = Cfg(
        B=B, S=S, D=D, F=F,
        use_bq=bool(np.any(bqkv[0:D])),
        use_bk=bool(np.any(bqkv[D : 2 * D])),
        use_bv=bool(np.any(bqkv[2 * D : 3 * D])),
        use_b1=bool(np.any(np.asarray(inputs["b1"]))),
        use_b2=bool(np.any(np.asarray(inputs["b2"]))),
        use_g1=not bool(np.all(np.asarray(inputs["g1"]) == 1.0)),
        use_bn1=bool(np.any(np.asarray(inputs["bn1"]))),
        use_g2=not bool(np.all(np.asarray(inputs["g2"]) == 1.0)),
        use_bn2=bool(np.any(np.asarray(inputs["bn2"]))),
    )
    return cfg


_NC_CACHE: dict = {}
_WEIGHTS: dict = {}

TRACE = False
LAST_RESULT = None


def kernel(**inputs) -> np.ndarray:
    global LAST_RESULT, _WEIGHTS
    cfg = make_cfg(inputs)
    w = prepare_weights(inputs, cfg)
    whash = hash(tuple(v.tobytes() for v in w.values()))
    key = (tuple(sorted(cfg.__dict__.items())), whash)
    if key not in _NC_CACHE:
        _WEIGHTS = w
        _NC_CACHE[key] = build_nc(cfg)
    nc = _NC_CACHE[key]

    in_maps, own_idx_per_core = host_prepare(inputs, cfg)
    ncores = 2 * cfg.B
    res = run_bass_kernel_spmd(
        nc, in_maps, core_ids=list(range(ncores)), trace=TRACE
    )
    LAST_RESULT = res

    out = np.empty((cfg.B, cfg.S, cfg.D), np.float32)
    for c in range(ncores):
        out[c // 2, own_idx_per_core[c]] = res.results[c]["out"].astype(np.float32)
    return out



# revision 33
# speedup vs baseline: 1.0056x; 1.0056x over previous
"""Trainium2 Bass kernel: causal transformer encoder layer, SPMD on 8 cores.

v2: fp8(e4m3) DoubleRow matmuls for QKV / out-proj / FFN / AV (k-tile pairs
contract in one MM at 0.5 cyc/row), bf16 scores, PE-side additive causal
masks (fp8 identity-matmul accumulated into score PSUM), residual adds via
scaled-identity matmuls, DMA-transpose for the FFN input transpose, and
LayerNorm rstd = exp(-0.5*ln(var)) so every ACT function lives in one table
set (natural_log_exp_and_others).

Sharding: core c handles batch b = c//2; the two cores of a batch split the
2048 query positions into 4 chunks of 256, paired so both halves have the
same static structure E = [16,12,8,4] k-tiles; per-core differences live in
mask DATA (additive 0/-30 biases on scores over the last two k-tile pairs).

fp8 scaling (power-of-2, exact): wq' = (Wq^T/8)*512, wk' = Wk^T*64,
wv' = Wv^T*64, wo' = Wo^T*64, w1' = W1^T*64, w2' = W2^T*64.  Evacuations
rescale Q by 2^-9, K/V by 2^-6 back to true scale.  The ones-column of Vaug
is 2^-6 so ctx comes out 64x true; out-proj PSUM is then 4096x true and the
residual enters via identity*4096 (bf16).  FFN: relu evac scales 2^-4 so hb
is 4x true, FFN2 PSUM is 256x true, residual via identity*256.  LayerNorm is
scale-invariant, so both LN outputs are true-scale.
"""

from contextlib import ExitStack
from dataclasses import dataclass

import numpy as np
import ml_dtypes

import concourse.bass as bass
import concourse.bacc as bacc
import concourse.tile as tile
from concourse import mybir
from concourse.bass_utils import run_bass_kernel_spmd
from concourse.masks import make_identity

F32 = mybir.dt.float32
F16 = mybir.dt.float16
BF16 = mybir.dt.bfloat16
F8 = mybir.dt.float8e4
AF = mybir.ActivationFunctionType
ALU = mybir.AluOpType

EPS = 1e-5
NEG = -30.0  # additive mask value (exp(-30) ~ 1e-13); exact in fp8e4m3
VPAD = 80    # per-head stride in Vaug (64 V + 1 ones + 15 pad, 16B aligned)


@dataclass
class Cfg:
    B: int = 4
    S: int = 2048
    D: int = 512
    F: int = 2048
    H: int = 8
    CHUNK: int = 256
    KT: int = 128
    use_bq: bool = False
    use_bk: bool = False
    use_bv: bool = False
    use_b1: bool = False
    use_b2: bool = False
    use_g1: bool = False
    use_bn1: bool = False
    use_g2: bool = False
    use_bn2: bool = False

    @property
    def HD(self):
        return self.D // self.H

    @property
    def DK(self):
        return self.D // 128

    @property
    def FK(self):
        return self.F // 128

    @property
    def NCH(self):
        return self.S // self.CHUNK

    @property
    def NQ(self):
        return (self.NCH // 2) * self.CHUNK

    @property
    def NSLOT(self):
        return self.NCH // 2

    @property
    def QT(self):
        return self.NQ // 128

    def ext(self, ci):
        return ((ci + 1) * self.CHUNK) // self.KT

    def slot_chunks(self, half):
        n = self.NCH
        if half == 0:
            s = [i for i in range(n) if i % 4 in (0, 3)]
        else:
            s = [i for i in range(n) if i % 4 in (1, 2)]
        return sorted(s, key=lambda ci: -self.ext(ci))

    def slot_qs(self, half):
        return [ci * self.CHUNK for ci in self.slot_chunks(half)]

    def slot_E(self):
        a = self.slot_chunks(0)
        b = self.slot_chunks(1)
        E = [max(self.ext(x), self.ext(y)) for x, y in zip(a, b)]
        for e in E:
            assert e % 4 == 0, E
        return E


def build_nc(cfg: Cfg, n_bodies: int = 1) -> bass.Bass:
    S, D, F, H, HD = cfg.S, cfg.D, cfg.F, cfg.H, cfg.HD
    DK, FK, QT, NQ, CHUNK, KT = cfg.DK, cfg.FK, cfg.QT, cfg.NQ, cfg.CHUNK, cfg.KT
    NSLOT = cfg.NSLOT
    E = cfg.slot_E()
    WQ = min(512, NQ)
    WS = min(512, S)

    nc = bacc.Bacc("TRN2", target_bir_lowering=False)

    xT_d = nc.declare_dram_parameter("xT", [D, S], F8, isOutput=False)
    # residual x for own tokens arrives as the fp8 QUANTIZATION ERROR of the
    # on-device-gathered xqT (plus bias folds); identity-matmul injects of
    # xqT + this error term reconstruct bf16-precision x in the out-proj PSUM.
    xoerr_d = nc.declare_dram_parameter("xoerr", [NQ, D], F8, isOutput=False)
    # weights are static across calls: embed as NEFF Const tensors (DMA'd to
    # HBM once at model load) instead of per-call staged inputs.  _WEIGHTS is
    # set by kernel() before build; the nc cache is keyed on its content.
    w = _WEIGHTS
    wqkv_d = nc.inline_tensor(w["wqkvT"], "wqkvT")
    wo_d = nc.inline_tensor(w["woT"], "woT")
    w1_d = nc.inline_tensor(w["w1T"], "w1T")
    w2_d = nc.inline_tensor(w["w2T"], "w2T")
    # only-if-used parameter declarations: absent params are absent inputs,
    # and host->device staging of the inputs dominates the e2e time.
    bq_d = nc.declare_dram_parameter("bq", [D], F32, isOutput=False) if cfg.use_bq else None
    bk_d = nc.declare_dram_parameter("bk", [D], F32, isOutput=False) if cfg.use_bk else None
    bv_d = nc.declare_dram_parameter("bv", [D], F32, isOutput=False) if cfg.use_bv else None
    b1_d = nc.declare_dram_parameter("b1", [F], F32, isOutput=False) if cfg.use_b1 else None
    b2_d = nc.declare_dram_parameter("b2", [128, D], F32, isOutput=False) if cfg.use_b2 else None
    g1_d = nc.declare_dram_parameter("g1v", [128, D], F32, isOutput=False) if cfg.use_g1 else None
    bn1_d = nc.declare_dram_parameter("bn1v", [128, D], F32, isOutput=False) if cfg.use_bn1 else None
    g2_d = nc.declare_dram_parameter("g2v", [128, D], F32, isOutput=False) if cfg.use_g2 else None
    bn2_d = nc.declare_dram_parameter("bn2v", [128, D], F32, isOutput=False) if cfg.use_bn2 else None
    # additive score masks for the last two k-tile pairs of each slot:
    # [128 keys, slot, pair-in-{T-2,T-1}, jj x 256 queries].  Content is
    # head-independent, so both hh matmuls read the same 512-col slice.
    # Masks are STRUCTURAL (depend only on shapes): bake both halves into
    # the NEFF and select by partition-id parity with predicated DMAs.
    masks0_d = nc.inline_tensor(build_masks(cfg, 0), "masks0")
    masks1_d = nc.inline_tensor(build_masks(cfg, 1), "masks1")
    out_d = nc.declare_dram_parameter("out", [NQ, D], F16, isOutput=True)

    with ExitStack() as top:
        tc = top.enter_context(tile.TileContext(nc, pool_alloc_mode="queue"))
        consts = top.enter_context(tc.tile_pool(name="consts", bufs=1))

        ident_f8 = consts.tile([128, 128], F8)
        make_identity(nc, ident_f8)
        ident_bf = consts.tile([128, 128], BF16)
        nc.gpsimd.memset(ident_bf, 0.0)
        nc.gpsimd.affine_select(
            out=ident_bf, in_=ident_bf, compare_op=ALU.not_equal,
            fill=1.0, base=0, pattern=[[-1, 128]], channel_multiplier=1,
        )
        ident128_f8 = consts.tile([128, 128], F8)
        nc.gpsimd.memset(ident128_f8, 0.0)
        nc.gpsimd.affine_select(
            out=ident128_f8, in_=ident128_f8, compare_op=ALU.not_equal,
            fill=128.0, base=0, pattern=[[-1, 128]], channel_multiplier=1,
        )
        ident256 = consts.tile([128, 128], BF16)
        nc.gpsimd.memset(ident256, 0.0)
        nc.gpsimd.affine_select(
            out=ident256, in_=ident256, compare_op=ALU.not_equal,
            fill=256.0, base=0, pattern=[[-1, 128]], channel_multiplier=1,
        )

        masks_sb = consts.tile([128, NSLOT, 2, 2 * CHUNK], F8)

        eps_sb = consts.tile([128, 1], F32)
        nc.vector.memset(eps_sb, EPS)

        bq_sb = bk_sb = bv_sb = b1_sb = None
        if cfg.use_bq:
            bq_sb = consts.tile([128, DK], F32)
            nc.sync.dma_start(out=bq_sb, in_=bq_d.rearrange("(m p) -> p m", p=128))
        if cfg.use_bk:
            bk_sb = consts.tile([128, DK], F32)
            nc.sync.dma_start(out=bk_sb, in_=bk_d.rearrange("(m p) -> p m", p=128))
        if cfg.use_bv:
            bv_sb = consts.tile([128, DK], F32)
            nc.sync.dma_start(out=bv_sb, in_=bv_d.rearrange("(m p) -> p m", p=128))
        if cfg.use_b1:
            b1_sb = consts.tile([128, FK], F32)
            nc.sync.dma_start(out=b1_sb, in_=b1_d.rearrange("(f p) -> p f", p=128))

        def bcast_const(dram, nm):
            t = consts.tile([128, D], F32, name=nm, tag=nm)
            nc.sync.dma_start(out=t, in_=dram[:, :])
            return t

        g1_b = bcast_const(g1_d, "g1b") if cfg.use_g1 else None
        bn1_b = bcast_const(bn1_d, "bn1b") if cfg.use_bn1 else None
        g2_b = bcast_const(g2_d, "g2b") if cfg.use_g2 else None
        bn2_b = bcast_const(bn2_d, "bn2b") if cfg.use_bn2 else None
        b2_b = bcast_const(b2_d, "b2b") if cfg.use_b2 else None

        def emit_body():
            pctx_cm = tc.tile_pool(name="pctx", bufs=1)
            pctx = pctx_cm.__enter__()
            # per-slot ctx tiles so out-proj can start as each slot finishes;
            # ctx_b is the bf16 token-major->feature-major transpose staging
            ctx_s = [pctx.tile([128, DK, CHUNK], F8, name=f"ctx{s}", tag=f"ctx{s}")
                     for s in range(NSLOT)]
            ctx_b = [pctx.tile([128, DK, CHUNK], BF16, name=f"ctb{s}", tag=f"ctb{s}")
                     for s in range(NSLOT)]

            pqkv_cm = tc.tile_pool(name="pqkv", bufs=1)
            pqkv = pqkv_cm.__enter__()

            # ---------------- Phase A: QKV projections (fp8 DoubleRow) -------
            pa_cm = tc.tile_pool(name="pa", bufs=1)
            pa = pa_cm.__enter__()

            NKP = DK // 2
            xT_sb = [pa.tile([128, 2, S], F8, name=f"xT{p}", tag=f"xT{p}")
                     for p in range(NKP)]
            # xqT lives in the long-lived pctx pool: phase C re-reads it to
            # reconstruct the bf16 residual x from fp8 + error term.
            xqT_sb = [pctx.tile([128, 2, NQ], F8, name=f"xq{p}", tag=f"xq{p}")
                      for p in range(NKP)]
            wqkv_sb = [pa.tile([128, 2, 3 * D], F8, name=f"wq{p}", tag=f"wq{p}")
                       for p in range(NKP)]
            xT_r = xT_d.rearrange("(m p) t -> p m t", p=128)
            wqkv_r = wqkv_d.rearrange("(m p) c -> p m c", p=128)
            # per-core query gather: the two halves of a batch need different
            # 256-column ranges of xT; predicated DMAs on partition-id parity
            # pick the right set with one shared program.
            pid = nc.scalar.partition_id()
            c_even = (pid & 1) == 0
            c_odd = (pid & 1) != 0
            for p in range(NKP):
                # issue on both HWDGE queues so the first Q inputs land
                # without serializing behind each other on SP
                for s in range(NSLOT):
                    for cond, qs in (
                        (c_even, cfg.slot_qs(0)[s]),
                        (c_odd, cfg.slot_qs(1)[s]),
                    ):
                        nc.scalar.dma_start(
                            out=xqT_sb[p][:, :, s * CHUNK : (s + 1) * CHUNK],
                            in_=xT_r[:, 2 * p : 2 * p + 2, qs : qs + CHUNK],
                            cond=cond,
                        )
                nc.sync.dma_start(out=wqkv_sb[p], in_=wqkv_r[:, 2 * p : 2 * p + 2, :])
            for p in range(NKP):
                nc.sync.dma_start(out=xT_sb[p], in_=xT_r[:, 2 * p : 2 * p + 2, :])
            nc.scalar.dma_start(out=masks_sb, in_=masks0_d[:, :, :, :], cond=c_even)
            nc.scalar.dma_start(out=masks_sb, in_=masks1_d[:, :, :, :], cond=c_odd)

            Qfm = [pqkv.tile([128, NQ], BF16, name=f"Qfm{m}", tag=f"Qfm{m}")
                   for m in range(DK)]
            Kfm = [pqkv.tile([128, S], BF16, name=f"Kfm{m}", tag=f"Kfm{m}")
                   for m in range(DK)]
            NVG = (S // KT) // 4
            Vaug = [pqkv.tile([128, 4, H * VPAD], F8, name=f"Va{g}", tag=f"Va{g}")
                    for g in range(NVG)]
            # ones columns (2^-6 so ctx comes out 64x true)
            for g in range(NVG):
                nc.vector.memset(
                    Vaug[g].rearrange("p t (h c) -> p t h c", h=H)[:, :, :, HD : HD + 1],
                    0.015625,
                )

            with tc.tile_pool(name="pa_psum", bufs=7, space="PSUM") as pap:
                def emit_q(m):
                    for ch in range(NQ // WQ):
                        ps = pap.tile([128, WQ], F32, tag="ps")
                        for kp in range(NKP):
                            nc.tensor.matmul(
                                out=ps,
                                lhsT=wqkv_sb[kp][:, :, m * 128 : (m + 1) * 128],
                                rhs=xqT_sb[kp][:, :, ch * WQ : (ch + 1) * WQ],
                                start=(kp == 0),
                                stop=(kp == NKP - 1),
                                perf_mode=mybir.MatmulPerfMode.DoubleRow,
                            )
                        dst = Qfm[m][:, ch * WQ : (ch + 1) * WQ]
                        nc.scalar.activation(
                            out=dst, in_=ps, func=AF.Identity,
                            bias=(bq_sb[:, m : m + 1] if cfg.use_bq else 0.0),
                            scale=1.0 / 512.0,
                        )

                def emit_k(ch):
                    for m in range(DK):
                        ps = pap.tile([128, WS], F32, tag="ps")
                        for kp in range(NKP):
                            nc.tensor.matmul(
                                out=ps,
                                lhsT=wqkv_sb[kp][:, :, D + m * 128 : D + (m + 1) * 128],
                                rhs=xT_sb[kp][:, :, ch * WS : (ch + 1) * WS],
                                start=(kp == 0),
                                stop=(kp == NKP - 1),
                                perf_mode=mybir.MatmulPerfMode.DoubleRow,
                            )
                        dst = Kfm[m][:, ch * WS : (ch + 1) * WS]
                        if cfg.use_bk:
                            nc.vector.tensor_scalar(
                                out=dst, in0=ps, scalar1=1.0 / 64.0,
                                scalar2=bk_sb[:, m : m + 1],
                                op0=ALU.mult, op1=ALU.add,
                            )
                        elif (ch * DK + m) % 2 == 0:
                            nc.scalar.activation(
                                out=dst, in_=ps, func=AF.Identity, scale=1.0 / 64.0
                            )
                        else:
                            nc.vector.tensor_scalar_mul(dst, ps, 1.0 / 64.0)

                def emit_v(g):
                    for t in range(4 * g, 4 * g + 4):
                        ps = pap.tile([128, D], F32, tag="ps")
                        for kp in range(NKP):
                            nc.tensor.matmul(
                                out=ps,
                                lhsT=xT_sb[kp][:, :, t * 128 : (t + 1) * 128],
                                rhs=wqkv_sb[kp][:, :, 2 * D : 3 * D],
                                start=(kp == 0),
                                stop=(kp == NKP - 1),
                                perf_mode=mybir.MatmulPerfMode.DoubleRow,
                            )
                        vdst = Vaug[t // 4][:, t % 4, :].rearrange("p (h c) -> p h c", h=H)
                        src = ps.rearrange("p (h c) -> p h c", h=H)
                        if t % 2 == 0:
                            nc.scalar.activation(
                                out=vdst[:, :, 0:HD], in_=src, func=AF.Identity,
                                scale=1.0 / 64.0,
                            )
                        else:
                            nc.vector.tensor_scalar_mul(
                                vdst[:, :, 0:HD], src, 1.0 / 64.0
                            )

                # interleave by attention consumption order (small slots first)
                emit_q(0)
                emit_k(0)
                emit_v(0)
                emit_q(1)
                emit_k(1)
                emit_v(1)
                emit_q(2)
                emit_q(3)
                emit_k(2)
                emit_v(2)
                emit_k(3)
                emit_v(3)

            pa_cm.__exit__(None, None, None)

            # ---------------- Phase B: attention -----------------------------
            with (
                tc.tile_pool(name="pb_sc", bufs=3, space="PSUM") as pbs,
                tc.tile_pool(name="pb_cx", bufs=2, space="PSUM") as pbc,
                tc.tile_pool(name="pb_es", bufs=5) as pbe,
                tc.tile_pool(name="pb_w", bufs=4) as pbw,
            ):
                sch_ctr = 0  # round-robin unmasked exp tiles between ACT/DVE
                slot_order = sorted(range(NSLOT), key=lambda s: E[s])
                for s in slot_order:
                    T = E[s] // 2
                    for m in range(DK):  # head pair (2m, 2m+1)
                        cps = pbc.tile([128, 2, 2, HD + 1], F32, tag="cps")
                        for t in range(T):
                            masked = t >= T - 2
                            sc = pbs.tile([128, 4 * CHUNK], F32, tag="sc")
                            for hh in range(2):
                                # each hh occupies one PSUM bank of sc; the
                                # FIRST matmul into the bank uses start=True
                                # (bank-wide has_written clear), all later
                                # ones start=False.
                                off = hh * HD
                                for jj in range(2):
                                    j = 2 * t + jj
                                    nc.tensor.matmul(
                                        out=sc[:, (2 * hh + jj) * CHUNK : (2 * hh + jj + 1) * CHUNK],
                                        lhsT=Kfm[m][off : off + HD, j * KT : (j + 1) * KT],
                                        rhs=Qfm[m][off : off + HD, s * CHUNK : (s + 1) * CHUNK],
                                        start=(jj == 0),
                                        stop=(jj == 1 and not masked),
                                        skip_group_check=True,
                                    )
                                if masked:
                                    # additive causal/equalization mask
                                    nc.tensor.matmul(
                                        out=sc[:, 2 * hh * CHUNK : (2 * hh + 2) * CHUNK],
                                        lhsT=ident_f8,
                                        rhs=masks_sb[:, s, t - (T - 2), :],
                                        start=False,
                                        stop=True,
                                        skip_group_check=True,
                                    )
                            es = pbe.tile([128, 4 * CHUNK], F8, tag="es")
                            # fp8 Schraudolph exp on DVE for ~29/80 tiles (the
                            # e4m3 bit pattern of exp(x) is ~ int8(8/ln2*x+56));
                            # unmasked scores are within +-1.5, and -30-masked
                            # scores saturate the int8 convert to -128 = -0.0
                            # in e4m3 (HW-verified), so every tile is eligible.
                            # Bresenham spread keeps ACT/DVE interleaved.
                            if (sch_ctr * 29) % 80 < 29:
                                nc.vector.tensor_scalar(
                                    out=es.bitcast(mybir.dt.int8),
                                    in0=sc,
                                    scalar1=11.5416,
                                    scalar2=56.0,
                                    op0=ALU.mult,
                                    op1=ALU.add,
                                )
                            else:
                                nc.scalar.activation(out=es, in_=sc, func=AF.Exp)
                            sch_ctr += 1
                            for hh in range(2):
                                # transposed AV: queries on partitions, 65 V
                                # columns (incl. 2^-6 ones) in the free dim —
                                # all four (qh, hh) streams accumulate in ONE
                                # PSUM bank; start=True only on the first MM.
                                h = 2 * m + hh
                                g, tg = (2 * t) // 4, (2 * t) % 4
                                es_hh = es[:, 2 * hh * CHUNK : 2 * (hh + 1) * CHUNK].rearrange(
                                    "p (two q) -> p two q", two=2
                                )
                                for qh in range(2):
                                    nc.tensor.matmul(
                                        out=cps[:, qh, hh, :],
                                        lhsT=es_hh[:, :, qh * 128 : (qh + 1) * 128],
                                        rhs=Vaug[g][:, tg : tg + 2, h * VPAD : h * VPAD + HD + 1],
                                        start=(t == 0 and hh == 0 and qh == 0),
                                        stop=(t == T - 1 and hh == 1 and qh == 1),
                                        perf_mode=mybir.MatmulPerfMode.DoubleRow,
                                        skip_group_check=True,
                                    )
                        inv = pbw.tile([128, 2, 2], F32, tag="inv")
                        for qh in range(2):
                            nc.vector.reciprocal(
                                out=inv[:, qh, :], in_=cps[:, qh, :, HD : HD + 1]
                            )
                        ctm = pbw.tile([128, 2, 128], BF16, tag="ctm")
                        for qh in range(2):
                            for hh in range(2):
                                nc.vector.tensor_scalar_mul(
                                    ctm[:, qh, hh * HD : (hh + 1) * HD],
                                    cps[:, qh, hh, 0:HD],
                                    inv[:, qh, hh : hh + 1],
                                )
                        for qh in range(2):
                            nc.sync.dma_start_transpose(
                                out=ctx_b[s][:, m, qh * 128 : (qh + 1) * 128],
                                in_=ctm[:, qh, :],
                            )
                    # all head-pairs transposed: widen to fp8 for out-proj
                    nc.gpsimd.tensor_copy(ctx_s[s], ctx_b[s])

            pqkv_cm.__exit__(None, None, None)

            # -------- late-weight loads (overlap attention) -------------------
            pc_cm = tc.tile_pool(name="pc", bufs=1)
            pc = pc_cm.__enter__()
            wo_sb = pc.tile([128, DK, D], F8)
            nc.sync.dma_start(out=wo_sb, in_=wo_d.rearrange("(m p) c -> p m c", p=128))
            xoerr_sb = pc.tile([128, QT, D], F8)
            nc.sync.dma_start(out=xoerr_sb, in_=xoerr_d.rearrange("(t p) d -> p t d", p=128))
            pw1_cm = tc.tile_pool(name="pw1", bufs=1)
            pw1 = pw1_cm.__enter__()
            w1_sb = pw1.tile([128, DK, F], F8)
            nc.sync.dma_start(out=w1_sb, in_=w1_d.rearrange("(m p) c -> p m c", p=128))

            # ---------------- Phase C: out-proj + LN1 + transpose -------------
            # token-tile halves so FFN chunk 1 can start mid-phase-C
            pd_cm = tc.tile_pool(name="pd", bufs=1)
            pd = pd_cm.__enter__()
            NCHD = NQ // WQ
            NSUB = WQ // 128
            xln1 = [pd.tile([128, NSUB, D], BF16, name=f"xl{c}", tag=f"xl{c}")
                    for c in range(NCHD)]
            x1tb = [pd.tile([128, DK, WQ], BF16, name=f"xb{c}", tag=f"xb{c}")
                    for c in range(NCHD)]
            x1t = [pd.tile([128, DK, WQ], F8, name=f"xf{c}", tag=f"xf{c}")
                   for c in range(NCHD)]

            pw2_cm = tc.tile_pool(name="pw2", bufs=1)
            pw2 = pw2_cm.__enter__()
            w2_sb = pw2.tile([128, FK, D], F8)
            nc.sync.dma_start(out=w2_sb, in_=w2_d.rearrange("(f p) c -> p f c", p=128))

            def layer_norm_step(tt, g_b, bn_b, dst, work, act_stats=False):
                # tt: [128, D] fp32 (PSUM ok); scale-invariant LN
                if act_stats:
                    # stats via ACT accum_out (sum / sum-of-squares reduced
                    # during the copy passes) — keeps DVE free in the tail
                    xraw = work.tile([128, D], F32, tag="xraw")
                    ssum = work.tile([128, 1], F32, tag="ssum")
                    nc.scalar.activation(
                        out=xraw, in_=tt, func=AF.Identity, accum_out=ssum
                    )
                    junk = work.tile([128, D], F32, tag="junk")
                    ssq = work.tile([128, 1], F32, tag="ssq")
                    nc.scalar.activation(
                        out=junk, in_=tt, func=AF.Square, accum_out=ssq
                    )
                    mu = work.tile([128, 1], F32, tag="mu")
                    nc.vector.tensor_scalar_mul(mu, ssum, 1.0 / D)
                    musq = work.tile([128, 1], F32, tag="musq")
                    nc.vector.tensor_mul(musq, mu, mu)
                    var = work.tile([128, 1], F32, tag="var")
                    nc.vector.tensor_scalar(
                        out=var, in0=ssq, scalar1=1.0 / D, scalar2=musq,
                        op0=ALU.mult, op1=ALU.subtract,
                    )
                    sd = work.tile([128, 1], F32, tag="sd")
                    nc.scalar.activation(out=sd, in_=var, func=AF.Sqrt, bias=eps_sb)
                    rstd = work.tile([128, 1], F32, tag="rstd")
                    nc.vector.reciprocal(out=rstd, in_=sd)
                    nc.vector.tensor_scalar(
                        out=dst, in0=xraw, scalar1=mu, scalar2=rstd,
                        op0=ALU.subtract, op1=ALU.mult,
                    )
                else:
                    stats = work.tile([128, nc.vector.BN_STATS_DIM], F32, tag="stats")
                    nc.vector.bn_stats(out=stats, in_=tt)
                    mv = work.tile([128, nc.vector.BN_AGGR_DIM], F32, tag="mv")
                    nc.vector.bn_aggr(out=mv, in_=stats)
                    sd = work.tile([128, 1], F32, tag="sd")
                    nc.scalar.activation(out=sd, in_=mv[:, 1:2], func=AF.Sqrt, bias=eps_sb)
                    rstd = work.tile([128, 1], F32, tag="rstd")
                    nc.vector.reciprocal(out=rstd, in_=sd)
                    nc.vector.tensor_scalar(
                        out=dst, in0=tt, scalar1=mv[:, 0:1], scalar2=rstd,
                        op0=ALU.subtract, op1=ALU.mult,
                    )
                if g_b is not None:
                    nc.vector.tensor_mul(dst, dst, g_b)
                if bn_b is not None:
                    nc.vector.tensor_add(dst, dst, bn_b)

            # process token tiles in the order their slots finish attention
            t_order = [2 * s + i for s in slot_order for i in range(2)]
            ch_order = []
            for t in t_order:
                if t // NSUB not in ch_order:
                    ch_order.append(t // NSUB)
            with (
                tc.tile_pool(name="pc_ps", bufs=4, space="PSUM") as pcp,
                tc.tile_pool(name="pc_w", bufs=3) as pcw,
            ):
                done = set()
                conv_done = set()
                for t in t_order:
                    ch, sub = t // NSUB, t % NSUB
                    ps = pcp.tile([128, D], F32, tag="ps")
                    for kp in range(DK // 2):
                        nc.tensor.matmul(
                            out=ps,
                            lhsT=ctx_s[t // 2][:, 2 * kp : 2 * kp + 2,
                                               (t % 2) * 128 : (t % 2 + 1) * 128],
                            rhs=wo_sb[:, 2 * kp : 2 * kp + 2, :],
                            start=(kp == 0),
                            stop=False,
                            perf_mode=mybir.MatmulPerfMode.DoubleRow,
                        )
                    # residual: out-proj PSUM is 128x true (wo' = 2*Wo^T).
                    # fp8 part: transpose-inject xqT blocks against 128*I;
                    # error part: token-major fp8 xoerr against 128*I.
                    # (128 is the largest power of 2 in f8e4's range, max 240.)
                    for m in range(DK):
                        nc.tensor.matmul(
                            out=ps[:, m * 128 : (m + 1) * 128],
                            lhsT=xqT_sb[m // 2][:, m % 2, t * 128 : (t + 1) * 128],
                            rhs=ident128_f8,
                            start=False,
                            stop=False,
                            skip_group_check=True,
                        )
                    nc.tensor.matmul(
                        out=ps,
                        lhsT=ident128_f8,
                        rhs=xoerr_sb[:, t, :],
                        start=False,
                        stop=True,
                        skip_group_check=True,
                    )
                    layer_norm_step(ps, g1_b, bn1_b, xln1[ch][:, sub, :], pcw)
                    for mm in range(DK):
                        nc.sync.dma_start_transpose(
                            out=x1tb[ch][:, mm, sub * 128 : (sub + 1) * 128],
                            in_=xln1[ch][:, sub, mm * 128 : (mm + 1) * 128],
                        )
                    done.add(t)
                    for c2 in range(NCHD):
                        if c2 not in conv_done and all(
                            c2 * NSUB + i in done for i in range(NSUB)
                        ):
                            conv_done.add(c2)
                            for mm in range(DK):
                                # bf16 -> fp8 on the otherwise-idle gpsimd
                                nc.gpsimd.tensor_copy(
                                    x1t[c2][:, mm, :], x1tb[c2][:, mm, :]
                                )

            # ---------------- Phase D: FFN + LN2 + store ----------------------
            # hb is split into per-f-pair tiles so FFN2 trails FFN1 at
            # f-granularity; FFN2 runs in sub-pairs with double-buffered yps
            # so the next chunk isn't blocked on LN2 draining.
            with (
                tc.tile_pool(name="pf_h", bufs=4, space="PSUM") as pfh,
                tc.tile_pool(name="pf_y", bufs=2, space="PSUM") as pfy,
                tc.tile_pool(name="pf_hb", bufs=3) as pfhb,
                tc.tile_pool(name="pf_w", bufs=3) as pfw,
                tc.tile_pool(name="pf_o", bufs=3) as pfo,
            ):
                for ch in ch_order:
                    hb_p = [pfhb.tile([128, 2, WQ], F8, name=f"hb{i}", tag=f"hb{i}")
                            for i in range(FK // 2)]
                    for f in range(FK):
                        hp = pfh.tile([128, WQ], F32, tag="hp")
                        for kp in range(DK // 2):
                            nc.tensor.matmul(
                                out=hp,
                                lhsT=w1_sb[:, 2 * kp : 2 * kp + 2, f * 128 : (f + 1) * 128],
                                rhs=x1t[ch][:, 2 * kp : 2 * kp + 2, :],
                                start=(kp == 0),
                                stop=(kp == DK // 2 - 1),
                                perf_mode=mybir.MatmulPerfMode.DoubleRow,
                            )
                        # hb = 4x true = relu(psum * 2^-4).  The last chunk's
                        # relus run in the tail where DVE is free: split 50/50
                        # there; earlier chunks keep ACT-light 1-in-4 on DVE.
                        hdst = hb_p[f // 2][:, f % 2, :]
                        dve_relu = f % 4 == 3
                        if cfg.use_b1:
                            nc.scalar.activation(
                                out=hdst, in_=hp, func=AF.Relu,
                                bias=b1_sb[:, f : f + 1], scale=1.0 / 16.0,
                            )
                        elif dve_relu:
                            nc.vector.tensor_scalar(
                                out=hdst, in0=hp,
                                scalar1=1.0 / 16.0, scalar2=0.0,
                                op0=ALU.mult, op1=ALU.max,
                            )
                        else:
                            nc.scalar.activation(
                                out=hdst, in_=hp, func=AF.Relu,
                                scale=1.0 / 16.0,
                            )
                    for half in range(NSUB // 2):
                        yps = [pfy.tile([128, D], F32, name=f"y{i}", tag=f"y{i}")
                               for i in range(2)]
                        for fp in range(FK // 2):
                            for s2 in range(2):
                                c0 = (2 * half + s2) * 128
                                nc.tensor.matmul(
                                    out=yps[s2],
                                    lhsT=hb_p[fp][:, :, c0 : c0 + 128],
                                    rhs=w2_sb[:, 2 * fp : 2 * fp + 2, :],
                                    start=(fp == 0),
                                    stop=False,
                                    perf_mode=mybir.MatmulPerfMode.DoubleRow,
                                )
                        for s2 in range(2):
                            sub = 2 * half + s2
                            t = ch * NSUB + sub
                            # residual: += 256 * xln1 (FFN2 PSUM is 256x true)
                            nc.tensor.matmul(
                                out=yps[s2],
                                lhsT=ident256,
                                rhs=xln1[ch][:, sub, :],
                                start=False,
                                stop=True,
                                skip_group_check=True,
                            )
                            ob = pfo.tile([128, D], F16, tag="ob")
                            # final-chunk LN2s: run one of each pair on ACT so
                            # the two chains go in parallel in the drain tail
                            layer_norm_step(
                                yps[s2], g2_b, bn2_b, ob, pfw,
                                act_stats=(ch == ch_order[-1] and s2 == 1),
                            )
                            if cfg.use_b2:
                                nc.vector.tensor_add(ob, ob, b2_b)
                            nc.sync.dma_start(
                                out=out_d[t * 128 : (t + 1) * 128, :], in_=ob
                            )

            pw2_cm.__exit__(None, None, None)
            pd_cm.__exit__(None, None, None)
            pw1_cm.__exit__(None, None, None)
            pc_cm.__exit__(None, None, None)
            pctx_cm.__exit__(None, None, None)

        for _ in range(n_bodies):
            emit_body()

    nc.compile()
    return nc


# ---------------------------------------------------------------------------
# host side
# ---------------------------------------------------------------------------

def build_masks(cfg: Cfg, half: int) -> np.ndarray:
    """Additive masks [128, NSLOT, 2, 2*CHUNK] fp8: 0 keep / NEG drop.

    Layout of last dim: (jj 2) x CHUNK queries (head-independent); tile
    index dim2 selects k-tile pair T-2 or T-1 of the slot.
    """
    E = cfg.slot_E()
    chunks = cfg.slot_chunks(half)
    qs_l = cfg.slot_qs(half)
    m = np.zeros((128, cfg.NSLOT, 2, 2 * cfg.CHUNK), np.float32)
    k_loc = np.arange(128)[:, None]
    q_loc = np.arange(cfg.CHUNK)[None, :]
    for s, (ci, qs) in enumerate(zip(chunks, qs_l)):
        T = E[s] // 2
        for ti, tpair in enumerate((T - 2, T - 1)):
            for jj in range(2):
                j = 2 * tpair + jj
                keep = (qs + q_loc) >= (j * cfg.KT + k_loc)
                bias = np.where(keep, 0.0, NEG)
                m[:, s, ti, jj * cfg.CHUNK : (jj + 1) * cfg.CHUNK] = bias
    return m.astype(ml_dtypes.float8_e4m3)


def prepare_weights(inputs: dict, cfg: Cfg) -> dict:
    """fp8-scaled transposed weight tensors baked into the NEFF as constants."""
    Wqkv = np.asarray(inputs["Wqkv"], np.float32)
    Wo = np.asarray(inputs["Wo"], np.float32)
    W1 = np.asarray(inputs["W1"], np.float32)
    W2 = np.asarray(inputs["W2"], np.float32)
    D = cfg.D
    scale = 1.0 / np.sqrt(np.float32(cfg.HD))
    f8 = ml_dtypes.float8_e4m3
    wqkvT = np.concatenate(
        [
            np.ascontiguousarray(Wqkv[0:D].T) * (scale * 512.0),
            np.ascontiguousarray(Wqkv[D : 2 * D].T) * 64.0,
            np.ascontiguousarray(Wqkv[2 * D : 3 * D].T) * 64.0,
        ],
        axis=1,
    )
    return {
        # wo' at 2x (not 64x) so the out-proj PSUM sits at 128x true scale,
        # letting the residual enter via fp8 128*I identity matmuls.
        "wqkvT": wqkvT.astype(f8),
        "woT": (np.ascontiguousarray(Wo.T) * 2.0).astype(f8),
        "w1T": (np.ascontiguousarray(W1.T) * 64.0).astype(f8),
        "w2T": (np.ascontiguousarray(W2.T) * 64.0).astype(f8),
    }


def host_prepare(inputs: dict, cfg: Cfg):
    x = np.asarray(inputs["x"], np.float32)
    bqkv = np.asarray(inputs["bqkv"], np.float32)
    Wo = np.asarray(inputs["Wo"], np.float32)
    bo = np.asarray(inputs["bo"], np.float32)
    b1 = np.asarray(inputs["b1"], np.float32)
    b2 = np.asarray(inputs["b2"], np.float32)
    g1 = np.asarray(inputs["g1"], np.float32)
    bn1 = np.asarray(inputs["bn1"], np.float32)
    g2 = np.asarray(inputs["g2"], np.float32)
    bn2 = np.asarray(inputs["bn2"], np.float32)

    D = cfg.D
    scale = 1.0 / np.sqrt(np.float32(cfg.HD))
    f8 = ml_dtypes.float8_e4m3
    bq = bqkv[0:D] * scale
    bk = bqkv[D : 2 * D]
    bv = bqkv[2 * D : 3 * D]

    in_maps = []
    own_idx_per_core = []
    for c in range(2 * cfg.B):
        b = c // 2
        half = c % 2
        own_idx = np.concatenate(
            [np.arange(qs, qs + cfg.CHUNK) for qs in cfg.slot_qs(half)]
        )
        own_idx_per_core.append(own_idx)
        xb = x[b]
        xown = xb[own_idx] + bo[None, :]
        if cfg.use_bv:
            # bv flows through attention as a constant added to every v;
            # since attention rows sum to 1, it adds bv @ Wo^T to attn_out.
            xown = xown + (bv @ Wo.T)[None, :]
        xq8 = xb[own_idx].astype(f8)
        im = {
            "xT": np.ascontiguousarray(xb.T).astype(f8),
            "xoerr": (xown - xq8.astype(np.float32)).astype(f8),
        }
        if cfg.use_bq:
            im["bq"] = bq
        if cfg.use_bk:
            im["bk"] = bk
        if cfg.use_bv:
            im["bv"] = bv
        if cfg.use_b1:
            im["b1"] = b1 * 4.0
        if cfg.use_b2:
            im["b2"] = np.tile(b2[None, :], (128, 1))
        if cfg.use_g1:
            im["g1v"] = np.tile(g1[None, :], (128, 1))
        if cfg.use_bn1:
            im["bn1v"] = np.tile(bn1[None, :], (128, 1))
        if cfg.use_g2:
            im["g2v"] = np.tile(g2[None, :], (128, 1))
        if cfg.use_bn2:
            im["bn2v"] = np.tile(bn2[None, :], (128, 1))
        in_maps.append(im)
    return in_maps, own_idx_per_core


def make_cfg(inputs: dict) -> Cfg:
    x = np.asarray(inputs["x"])
    B, S, D = x.shape
    F = np.asarray(inputs["W1"]).shape[0]
    bqkv = np.asarray(inputs["bqkv"], np.float32)
    cfg = Cfg(
        B=B, S=S, D=D, F=F,
        use_bq=bool(np.any(bqkv[0:D])),
        use_bk=bool(np.any(bqkv[D : 2 * D])),
        use_bv=bool(np.any(bqkv[2 * D : 3 * D])),
        use_b1=bool(np.any(np.asarray(inputs["b1"]))),
        use_b2=bool(np.any(np.asarray(inputs["b2"]))),
        use_g1=not bool(np.all(np.asarray(inputs["g1"]) == 1.0)),
        use_bn1=bool(np.any(np.asarray(inputs["bn1"]))),
        use_g2=not bool(np.all(np.asarray(inputs["g2"]) == 1.0)),
        use_bn2=bool(np.any(np.asarray(inputs["bn2"]))),
    )
    return cfg


_NC_CACHE: dict = {}
_WEIGHTS: dict = {}

TRACE = False
LAST_RESULT = None


def kernel(**inputs) -> np.ndarray:
    global LAST_RESULT, _WEIGHTS
    cfg = make_cfg(inputs)
    w = prepare_weights(inputs, cfg)
    whash = hash(tuple(v.tobytes() for v in w.values()))
    key = (tuple(sorted(cfg.__dict__.items())), whash)
    if key not in _NC_CACHE:
        _WEIGHTS = w
        _NC_CACHE[key] = build_nc(cfg)
    nc = _NC_CACHE[key]

    in_maps, own_idx_per_core = host_prepare(inputs, cfg)
    ncores = 2 * cfg.B
    res = run_bass_kernel_spmd(
        nc, in_maps, core_ids=list(range(ncores)), trace=TRACE
    )
    LAST_RESULT = res

    out = np.empty((cfg.B, cfg.S, cfg.D), np.float32)
    for c in range(ncores):
        out[c // 2, own_idx_per_core[c]] = res.results[c]["out"].astype(np.float32)
    return out



# revision 37
# speedup vs baseline: 98.2392x; 97.6928x over previous
"""Trainium2 Bass kernel: causal transformer encoder layer, SPMD on 8 cores.

v2: fp8(e4m3) DoubleRow matmuls for QKV / out-proj / FFN / AV (k-tile pairs
contract in one MM at 0.5 cyc/row), bf16 scores, PE-side additive causal
masks (fp8 identity-matmul accumulated into score PSUM), residual adds via
scaled-identity matmuls, DMA-transpose for the FFN input transpose, and
LayerNorm rstd = exp(-0.5*ln(var)) so every ACT function lives in one table
set (natural_log_exp_and_others).

Sharding: core c handles batch b = c//2; the two cores of a batch split the
2048 query positions into 4 chunks of 256, paired so both halves have the
same static structure E = [16,12,8,4] k-tiles; per-core differences live in
mask DATA (additive 0/-30 biases on scores over the last two k-tile pairs).

fp8 scaling (power-of-2, exact): wq' = (Wq^T/8)*512, wk' = Wk^T*64,
wv' = Wv^T*64, wo' = Wo^T*64, w1' = W1^T*64, w2' = W2^T*64.  Evacuations
rescale Q by 2^-9, K/V by 2^-6 back to true scale.  The ones-column of Vaug
is 2^-6 so ctx comes out 64x true; out-proj PSUM is then 4096x true and the
residual enters via identity*4096 (bf16).  FFN: relu evac scales 2^-4 so hb
is 4x true, FFN2 PSUM is 256x true, residual via identity*256.  LayerNorm is
scale-invariant, so both LN outputs are true-scale.
"""

from contextlib import ExitStack
from dataclasses import dataclass

import numpy as np
import ml_dtypes

import concourse.bass as bass
import concourse.bacc as bacc
import concourse.tile as tile
from concourse import mybir
from concourse.bass_utils import run_bass_kernel_spmd
from concourse.masks import make_identity

F32 = mybir.dt.float32
F16 = mybir.dt.float16
BF16 = mybir.dt.bfloat16
F8 = mybir.dt.float8e4
AF = mybir.ActivationFunctionType
ALU = mybir.AluOpType

EPS = 1e-5
NEG = -30.0  # additive mask value (exp(-30) ~ 1e-13); exact in fp8e4m3
VPAD = 80    # per-head stride in Vaug (64 V + 1 ones + 15 pad, 16B aligned)


@dataclass
class Cfg:
    B: int = 4
    S: int = 2048
    D: int = 512
    F: int = 2048
    H: int = 8
    CHUNK: int = 256
    KT: int = 128
    use_bq: bool = False
    use_bk: bool = False
    use_bv: bool = False
    use_b1: bool = False
    use_b2: bool = False
    use_g1: bool = False
    use_bn1: bool = False
    use_g2: bool = False
    use_bn2: bool = False

    @property
    def HD(self):
        return self.D // self.H

    @property
    def DK(self):
        return self.D // 128

    @property
    def FK(self):
        return self.F // 128

    @property
    def NCH(self):
        return self.S // self.CHUNK

    @property
    def NQ(self):
        return (self.NCH // 2) * self.CHUNK

    @property
    def NSLOT(self):
        return self.NCH // 2

    @property
    def QT(self):
        return self.NQ // 128

    def ext(self, ci):
        return ((ci + 1) * self.CHUNK) // self.KT

    def slot_chunks(self, half):
        n = self.NCH
        if half == 0:
            s = [i for i in range(n) if i % 4 in (0, 3)]
        else:
            s = [i for i in range(n) if i % 4 in (1, 2)]
        return sorted(s, key=lambda ci: -self.ext(ci))

    def slot_qs(self, half):
        return [ci * self.CHUNK for ci in self.slot_chunks(half)]

    def slot_E(self):
        a = self.slot_chunks(0)
        b = self.slot_chunks(1)
        E = [max(self.ext(x), self.ext(y)) for x, y in zip(a, b)]
        for e in E:
            assert e % 4 == 0, E
        return E


def build_nc(cfg: Cfg, n_bodies: int = 1) -> bass.Bass:
    S, D, F, H, HD = cfg.S, cfg.D, cfg.F, cfg.H, cfg.HD
    DK, FK, QT, NQ, CHUNK, KT = cfg.DK, cfg.FK, cfg.QT, cfg.NQ, cfg.CHUNK, cfg.KT
    NSLOT = cfg.NSLOT
    E = cfg.slot_E()
    WQ = min(512, NQ)
    WS = min(512, S)

    nc = bacc.Bacc("TRN2", target_bir_lowering=False)

    xT_d = nc.declare_dram_parameter("xT", [D, S], F8, isOutput=False)
    # residual x for own tokens arrives as the fp8 QUANTIZATION ERROR of the
    # on-device-gathered xqT (plus bias folds); identity-matmul injects of
    # xqT + this error term reconstruct bf16-precision x in the out-proj PSUM.
    xoerr_d = nc.declare_dram_parameter("xoerr", [NQ, D], F8, isOutput=False)
    # weights are static across calls: embed as NEFF Const tensors (DMA'd to
    # HBM once at model load) instead of per-call staged inputs.  _WEIGHTS is
    # set by kernel() before build; the nc cache is keyed on its content.
    w = _WEIGHTS
    wqkv_d = nc.inline_tensor(w["wqkvT"], "wqkvT")
    wo_d = nc.inline_tensor(w["woT"], "woT")
    w1_d = nc.inline_tensor(w["w1T"], "w1T")
    w2_d = nc.inline_tensor(w["w2T"], "w2T")
    # only-if-used parameter declarations: absent params are absent inputs,
    # and host->device staging of the inputs dominates the e2e time.
    bq_d = nc.declare_dram_parameter("bq", [D], F32, isOutput=False) if cfg.use_bq else None
    bk_d = nc.declare_dram_parameter("bk", [D], F32, isOutput=False) if cfg.use_bk else None
    bv_d = nc.declare_dram_parameter("bv", [D], F32, isOutput=False) if cfg.use_bv else None
    b1_d = nc.declare_dram_parameter("b1", [F], F32, isOutput=False) if cfg.use_b1 else None
    b2_d = nc.declare_dram_parameter("b2", [128, D], F32, isOutput=False) if cfg.use_b2 else None
    g1_d = nc.declare_dram_parameter("g1v", [128, D], F32, isOutput=False) if cfg.use_g1 else None
    bn1_d = nc.declare_dram_parameter("bn1v", [128, D], F32, isOutput=False) if cfg.use_bn1 else None
    g2_d = nc.declare_dram_parameter("g2v", [128, D], F32, isOutput=False) if cfg.use_g2 else None
    bn2_d = nc.declare_dram_parameter("bn2v", [128, D], F32, isOutput=False) if cfg.use_bn2 else None
    # additive score masks for the last two k-tile pairs of each slot:
    # [128 keys, slot, pair-in-{T-2,T-1}, jj x 256 queries].  Content is
    # head-independent, so both hh matmuls read the same 512-col slice.
    # Masks are STRUCTURAL (depend only on shapes): bake both halves into
    # the NEFF and select by partition-id parity with predicated DMAs.
    masks0_d = nc.inline_tensor(build_masks(cfg, 0), "masks0")
    masks1_d = nc.inline_tensor(build_masks(cfg, 1), "masks1")
    out_d = nc.declare_dram_parameter("out", [NQ, D], F16, isOutput=True)

    with ExitStack() as top:
        tc = top.enter_context(tile.TileContext(nc, pool_alloc_mode="queue"))
        consts = top.enter_context(tc.tile_pool(name="consts", bufs=1))

        ident_f8 = consts.tile([128, 128], F8)
        make_identity(nc, ident_f8)
        ident_bf = consts.tile([128, 128], BF16)
        nc.gpsimd.memset(ident_bf, 0.0)
        nc.gpsimd.affine_select(
            out=ident_bf, in_=ident_bf, compare_op=ALU.not_equal,
            fill=1.0, base=0, pattern=[[-1, 128]], channel_multiplier=1,
        )
        ident128_f8 = consts.tile([128, 128], F8)
        nc.gpsimd.memset(ident128_f8, 0.0)
        nc.gpsimd.affine_select(
            out=ident128_f8, in_=ident128_f8, compare_op=ALU.not_equal,
            fill=128.0, base=0, pattern=[[-1, 128]], channel_multiplier=1,
        )
        ident256 = consts.tile([128, 128], BF16)
        nc.gpsimd.memset(ident256, 0.0)
        nc.gpsimd.affine_select(
            out=ident256, in_=ident256, compare_op=ALU.not_equal,
            fill=256.0, base=0, pattern=[[-1, 128]], channel_multiplier=1,
        )

        masks_sb = consts.tile([128, NSLOT, 2, 2 * CHUNK], F8)

        eps_sb = consts.tile([128, 1], F32)
        nc.vector.memset(eps_sb, EPS)

        bq_sb = bk_sb = bv_sb = b1_sb = None
        if cfg.use_bq:
            bq_sb = consts.tile([128, DK], F32)
            nc.sync.dma_start(out=bq_sb, in_=bq_d.rearrange("(m p) -> p m", p=128))
        if cfg.use_bk:
            bk_sb = consts.tile([128, DK], F32)
            nc.sync.dma_start(out=bk_sb, in_=bk_d.rearrange("(m p) -> p m", p=128))
        if cfg.use_bv:
            bv_sb = consts.tile([128, DK], F32)
            nc.sync.dma_start(out=bv_sb, in_=bv_d.rearrange("(m p) -> p m", p=128))
        if cfg.use_b1:
            b1_sb = consts.tile([128, FK], F32)
            nc.sync.dma_start(out=b1_sb, in_=b1_d.rearrange("(f p) -> p f", p=128))

        def bcast_const(dram, nm):
            t = consts.tile([128, D], F32, name=nm, tag=nm)
            nc.sync.dma_start(out=t, in_=dram[:, :])
            return t

        g1_b = bcast_const(g1_d, "g1b") if cfg.use_g1 else None
        bn1_b = bcast_const(bn1_d, "bn1b") if cfg.use_bn1 else None
        g2_b = bcast_const(g2_d, "g2b") if cfg.use_g2 else None
        bn2_b = bcast_const(bn2_d, "bn2b") if cfg.use_bn2 else None
        b2_b = bcast_const(b2_d, "b2b") if cfg.use_b2 else None

        def emit_body():
            pctx_cm = tc.tile_pool(name="pctx", bufs=1)
            pctx = pctx_cm.__enter__()
            # per-slot ctx tiles so out-proj can start as each slot finishes;
            # ctx_b is the bf16 token-major->feature-major transpose staging
            ctx_s = [pctx.tile([128, DK, CHUNK], F8, name=f"ctx{s}", tag=f"ctx{s}")
                     for s in range(NSLOT)]
            ctx_b = [pctx.tile([128, DK, CHUNK], BF16, name=f"ctb{s}", tag=f"ctb{s}")
                     for s in range(NSLOT)]

            pqkv_cm = tc.tile_pool(name="pqkv", bufs=1)
            pqkv = pqkv_cm.__enter__()

            # ---------------- Phase A: QKV projections (fp8 DoubleRow) -------
            pa_cm = tc.tile_pool(name="pa", bufs=1)
            pa = pa_cm.__enter__()

            NKP = DK // 2
            xT_sb = [pa.tile([128, 2, S], F8, name=f"xT{p}", tag=f"xT{p}")
                     for p in range(NKP)]
            # xqT lives in the long-lived pctx pool: phase C re-reads it to
            # reconstruct the bf16 residual x from fp8 + error term.
            xqT_sb = [pctx.tile([128, 2, NQ], F8, name=f"xq{p}", tag=f"xq{p}")
                      for p in range(NKP)]
            wqkv_sb = [pa.tile([128, 2, 3 * D], F8, name=f"wq{p}", tag=f"wq{p}")
                       for p in range(NKP)]
            xT_r = xT_d.rearrange("(m p) t -> p m t", p=128)
            wqkv_r = wqkv_d.rearrange("(m p) c -> p m c", p=128)
            # per-core query gather: the two halves of a batch need different
            # 256-column ranges of xT; predicated DMAs on partition-id parity
            # pick the right set with one shared program.
            pid = nc.scalar.partition_id()
            c_even = (pid & 1) == 0
            c_odd = (pid & 1) != 0
            for p in range(NKP):
                # issue on both HWDGE queues so the first Q inputs land
                # without serializing behind each other on SP
                for s in range(NSLOT):
                    for cond, qs in (
                        (c_even, cfg.slot_qs(0)[s]),
                        (c_odd, cfg.slot_qs(1)[s]),
                    ):
                        nc.scalar.dma_start(
                            out=xqT_sb[p][:, :, s * CHUNK : (s + 1) * CHUNK],
                            in_=xT_r[:, 2 * p : 2 * p + 2, qs : qs + CHUNK],
                            cond=cond,
                        )
                nc.sync.dma_start(out=wqkv_sb[p], in_=wqkv_r[:, 2 * p : 2 * p + 2, :])
            for p in range(NKP):
                nc.sync.dma_start(out=xT_sb[p], in_=xT_r[:, 2 * p : 2 * p + 2, :])
            nc.scalar.dma_start(out=masks_sb, in_=masks0_d[:, :, :, :], cond=c_even)
            nc.scalar.dma_start(out=masks_sb, in_=masks1_d[:, :, :, :], cond=c_odd)

            Qfm = [pqkv.tile([128, NQ], BF16, name=f"Qfm{m}", tag=f"Qfm{m}")
                   for m in range(DK)]
            Kfm = [pqkv.tile([128, S], BF16, name=f"Kfm{m}", tag=f"Kfm{m}")
                   for m in range(DK)]
            NVG = (S // KT) // 4
            Vaug = [pqkv.tile([128, 4, H * VPAD], F8, name=f"Va{g}", tag=f"Va{g}")
                    for g in range(NVG)]
            # ones columns (2^-6 so ctx comes out 64x true)
            for g in range(NVG):
                nc.vector.memset(
                    Vaug[g].rearrange("p t (h c) -> p t h c", h=H)[:, :, :, HD : HD + 1],
                    0.015625,
                )

            with tc.tile_pool(name="pa_psum", bufs=7, space="PSUM") as pap:
                def emit_q(m):
                    for ch in range(NQ // WQ):
                        ps = pap.tile([128, WQ], F32, tag="ps")
                        for kp in range(NKP):
                            nc.tensor.matmul(
                                out=ps,
                                lhsT=wqkv_sb[kp][:, :, m * 128 : (m + 1) * 128],
                                rhs=xqT_sb[kp][:, :, ch * WQ : (ch + 1) * WQ],
                                start=(kp == 0),
                                stop=(kp == NKP - 1),
                                perf_mode=mybir.MatmulPerfMode.DoubleRow,
                            )
                        dst = Qfm[m][:, ch * WQ : (ch + 1) * WQ]
                        nc.scalar.activation(
                            out=dst, in_=ps, func=AF.Identity,
                            bias=(bq_sb[:, m : m + 1] if cfg.use_bq else 0.0),
                            scale=1.0 / 512.0,
                        )

                def emit_k(ch):
                    for m in range(DK):
                        ps = pap.tile([128, WS], F32, tag="ps")
                        for kp in range(NKP):
                            nc.tensor.matmul(
                                out=ps,
                                lhsT=wqkv_sb[kp][:, :, D + m * 128 : D + (m + 1) * 128],
                                rhs=xT_sb[kp][:, :, ch * WS : (ch + 1) * WS],
                                start=(kp == 0),
                                stop=(kp == NKP - 1),
                                perf_mode=mybir.MatmulPerfMode.DoubleRow,
                            )
                        dst = Kfm[m][:, ch * WS : (ch + 1) * WS]
                        if cfg.use_bk:
                            nc.vector.tensor_scalar(
                                out=dst, in0=ps, scalar1=1.0 / 64.0,
                                scalar2=bk_sb[:, m : m + 1],
                                op0=ALU.mult, op1=ALU.add,
                            )
                        elif (ch * DK + m) % 2 == 0:
                            nc.scalar.activation(
                                out=dst, in_=ps, func=AF.Identity, scale=1.0 / 64.0
                            )
                        else:
                            nc.vector.tensor_scalar_mul(dst, ps, 1.0 / 64.0)

                def emit_v(g):
                    for t in range(4 * g, 4 * g + 4):
                        ps = pap.tile([128, D], F32, tag="ps")
                        for kp in range(NKP):
                            nc.tensor.matmul(
                                out=ps,
                                lhsT=xT_sb[kp][:, :, t * 128 : (t + 1) * 128],
                                rhs=wqkv_sb[kp][:, :, 2 * D : 3 * D],
                                start=(kp == 0),
                                stop=(kp == NKP - 1),
                                perf_mode=mybir.MatmulPerfMode.DoubleRow,
                            )
                        vdst = Vaug[t // 4][:, t % 4, :].rearrange("p (h c) -> p h c", h=H)
                        src = ps.rearrange("p (h c) -> p h c", h=H)
                        if t % 2 == 0:
                            nc.scalar.activation(
                                out=vdst[:, :, 0:HD], in_=src, func=AF.Identity,
                                scale=1.0 / 64.0,
                            )
                        else:
                            nc.vector.tensor_scalar_mul(
                                vdst[:, :, 0:HD], src, 1.0 / 64.0
                            )

                # interleave by attention consumption order (small slots first)
                emit_q(0)
                emit_k(0)
                emit_v(0)
                emit_q(1)
                emit_k(1)
                emit_v(1)
                emit_q(2)
                emit_q(3)
                emit_k(2)
                emit_v(2)
                emit_k(3)
                emit_v(3)

            pa_cm.__exit__(None, None, None)

            # ---------------- Phase B: attention -----------------------------
            with (
                tc.tile_pool(name="pb_sc", bufs=3, space="PSUM") as pbs,
                tc.tile_pool(name="pb_cx", bufs=2, space="PSUM") as pbc,
                tc.tile_pool(name="pb_es", bufs=5) as pbe,
                tc.tile_pool(name="pb_w", bufs=4) as pbw,
            ):
                sch_ctr = 0  # round-robin unmasked exp tiles between ACT/DVE
                slot_order = sorted(range(NSLOT), key=lambda s: E[s])
                for s in slot_order:
                    T = E[s] // 2
                    for m in range(DK):  # head pair (2m, 2m+1)
                        cps = pbc.tile([128, 2, 2, HD + 1], F32, tag="cps")
                        for t in range(T):
                            masked = t >= T - 2
                            sc = pbs.tile([128, 4 * CHUNK], F32, tag="sc")
                            for hh in range(2):
                                # each hh occupies one PSUM bank of sc; the
                                # FIRST matmul into the bank uses start=True
                                # (bank-wide has_written clear), all later
                                # ones start=False.
                                off = hh * HD
                                for jj in range(2):
                                    j = 2 * t + jj
                                    nc.tensor.matmul(
                                        out=sc[:, (2 * hh + jj) * CHUNK : (2 * hh + jj + 1) * CHUNK],
                                        lhsT=Kfm[m][off : off + HD, j * KT : (j + 1) * KT],
                                        rhs=Qfm[m][off : off + HD, s * CHUNK : (s + 1) * CHUNK],
                                        start=(jj == 0),
                                        stop=(jj == 1 and not masked),
                                        skip_group_check=True,
                                    )
                                if masked:
                                    # additive causal/equalization mask
                                    nc.tensor.matmul(
                                        out=sc[:, 2 * hh * CHUNK : (2 * hh + 2) * CHUNK],
                                        lhsT=ident_f8,
                                        rhs=masks_sb[:, s, t - (T - 2), :],
                                        start=False,
                                        stop=True,
                                        skip_group_check=True,
                                    )
                            es = pbe.tile([128, 4 * CHUNK], F8, tag="es")
                            # fp8 Schraudolph exp on DVE for ~29/80 tiles (the
                            # e4m3 bit pattern of exp(x) is ~ int8(8/ln2*x+56));
                            # unmasked scores are within +-1.5, and -30-masked
                            # scores saturate the int8 convert to -128 = -0.0
                            # in e4m3 (HW-verified), so every tile is eligible.
                            # Bresenham spread keeps ACT/DVE interleaved.
                            if (sch_ctr * 29) % 80 < 29:
                                nc.vector.tensor_scalar(
                                    out=es.bitcast(mybir.dt.int8),
                                    in0=sc,
                                    scalar1=11.5416,
                                    scalar2=56.0,
                                    op0=ALU.mult,
                                    op1=ALU.add,
                                )
                            else:
                                nc.scalar.activation(out=es, in_=sc, func=AF.Exp)
                            sch_ctr += 1
                            for hh in range(2):
                                # transposed AV: queries on partitions, 65 V
                                # columns (incl. 2^-6 ones) in the free dim —
                                # all four (qh, hh) streams accumulate in ONE
                                # PSUM bank; start=True only on the first MM.
                                h = 2 * m + hh
                                g, tg = (2 * t) // 4, (2 * t) % 4
                                es_hh = es[:, 2 * hh * CHUNK : 2 * (hh + 1) * CHUNK].rearrange(
                                    "p (two q) -> p two q", two=2
                                )
                                for qh in range(2):
                                    nc.tensor.matmul(
                                        out=cps[:, qh, hh, :],
                                        lhsT=es_hh[:, :, qh * 128 : (qh + 1) * 128],
                                        rhs=Vaug[g][:, tg : tg + 2, h * VPAD : h * VPAD + HD + 1],
                                        start=(t == 0 and hh == 0 and qh == 0),
                                        stop=(t == T - 1 and hh == 1 and qh == 1),
                                        perf_mode=mybir.MatmulPerfMode.DoubleRow,
                                        skip_group_check=True,
                                    )
                        inv = pbw.tile([128, 2, 2], F32, tag="inv")
                        for qh in range(2):
                            nc.vector.reciprocal(
                                out=inv[:, qh, :], in_=cps[:, qh, :, HD : HD + 1]
                            )
                        ctm = pbw.tile([128, 2, 128], BF16, tag="ctm")
                        for qh in range(2):
                            for hh in range(2):
                                nc.vector.tensor_scalar_mul(
                                    ctm[:, qh, hh * HD : (hh + 1) * HD],
                                    cps[:, qh, hh, 0:HD],
                                    inv[:, qh, hh : hh + 1],
                                )
                        for qh in range(2):
                            nc.sync.dma_start_transpose(
                                out=ctx_b[s][:, m, qh * 128 : (qh + 1) * 128],
                                in_=ctm[:, qh, :],
                            )
                    # all head-pairs transposed: widen to fp8 for out-proj
                    nc.gpsimd.tensor_copy(ctx_s[s], ctx_b[s])

            pqkv_cm.__exit__(None, None, None)

            # -------- late-weight loads (overlap attention) -------------------
            pc_cm = tc.tile_pool(name="pc", bufs=1)
            pc = pc_cm.__enter__()
            wo_sb = pc.tile([128, DK, D], F8)
            nc.sync.dma_start(out=wo_sb, in_=wo_d.rearrange("(m p) c -> p m c", p=128))
            xoerr_sb = pc.tile([128, QT, D], F8)
            nc.sync.dma_start(out=xoerr_sb, in_=xoerr_d.rearrange("(t p) d -> p t d", p=128))
            pw1_cm = tc.tile_pool(name="pw1", bufs=1)
            pw1 = pw1_cm.__enter__()
            w1_sb = pw1.tile([128, DK, F], F8)
            nc.sync.dma_start(out=w1_sb, in_=w1_d.rearrange("(m p) c -> p m c", p=128))

            # ---------------- Phase C: out-proj + LN1 + transpose -------------
            # token-tile halves so FFN chunk 1 can start mid-phase-C
            pd_cm = tc.tile_pool(name="pd", bufs=1)
            pd = pd_cm.__enter__()
            NCHD = NQ // WQ
            NSUB = WQ // 128
            xln1 = [pd.tile([128, NSUB, D], BF16, name=f"xl{c}", tag=f"xl{c}")
                    for c in range(NCHD)]
            x1t = [pd.tile([128, DK, WQ], F8, name=f"xf{c}", tag=f"xf{c}")
                   for c in range(NCHD)]

            pw2_cm = tc.tile_pool(name="pw2", bufs=1)
            pw2 = pw2_cm.__enter__()
            w2_sb = pw2.tile([128, FK, D], F8)
            nc.sync.dma_start(out=w2_sb, in_=w2_d.rearrange("(f p) c -> p f c", p=128))

            def layer_norm_step(tt, g_b, bn_b, dst, work, act_stats=False):
                # tt: [128, D] fp32 (PSUM ok); scale-invariant LN
                if act_stats:
                    # stats via ACT accum_out (sum / sum-of-squares reduced
                    # during the copy passes) — keeps DVE free in the tail
                    xraw = work.tile([128, D], F32, tag="xraw")
                    ssum = work.tile([128, 1], F32, tag="ssum")
                    nc.scalar.activation(
                        out=xraw, in_=tt, func=AF.Identity, accum_out=ssum
                    )
                    junk = work.tile([128, D], F32, tag="junk")
                    ssq = work.tile([128, 1], F32, tag="ssq")
                    nc.scalar.activation(
                        out=junk, in_=tt, func=AF.Square, accum_out=ssq
                    )
                    mu = work.tile([128, 1], F32, tag="mu")
                    nc.vector.tensor_scalar_mul(mu, ssum, 1.0 / D)
                    musq = work.tile([128, 1], F32, tag="musq")
                    nc.vector.tensor_mul(musq, mu, mu)
                    var = work.tile([128, 1], F32, tag="var")
                    nc.vector.tensor_scalar(
                        out=var, in0=ssq, scalar1=1.0 / D, scalar2=musq,
                        op0=ALU.mult, op1=ALU.subtract,
                    )
                    sd = work.tile([128, 1], F32, tag="sd")
                    nc.scalar.activation(out=sd, in_=var, func=AF.Sqrt, bias=eps_sb)
                    rstd = work.tile([128, 1], F32, tag="rstd")
                    nc.vector.reciprocal(out=rstd, in_=sd)
                    nc.vector.tensor_scalar(
                        out=dst, in0=xraw, scalar1=mu, scalar2=rstd,
                        op0=ALU.subtract, op1=ALU.mult,
                    )
                else:
                    stats = work.tile([128, nc.vector.BN_STATS_DIM], F32, tag="stats")
                    nc.vector.bn_stats(out=stats, in_=tt)
                    mv = work.tile([128, nc.vector.BN_AGGR_DIM], F32, tag="mv")
                    nc.vector.bn_aggr(out=mv, in_=stats)
                    sd = work.tile([128, 1], F32, tag="sd")
                    nc.scalar.activation(out=sd, in_=mv[:, 1:2], func=AF.Sqrt, bias=eps_sb)
                    rstd = work.tile([128, 1], F32, tag="rstd")
                    nc.vector.reciprocal(out=rstd, in_=sd)
                    nc.vector.tensor_scalar(
                        out=dst, in0=tt, scalar1=mv[:, 0:1], scalar2=rstd,
                        op0=ALU.subtract, op1=ALU.mult,
                    )
                if g_b is not None:
                    nc.vector.tensor_mul(dst, dst, g_b)
                if bn_b is not None:
                    nc.vector.tensor_add(dst, dst, bn_b)

            # process token tiles in the order their slots finish attention
            t_order = [2 * s + i for s in slot_order for i in range(2)]
            ch_order = []
            for t in t_order:
                if t // NSUB not in ch_order:
                    ch_order.append(t // NSUB)
            with (
                tc.tile_pool(name="pc_ps", bufs=4, space="PSUM") as pcp,
                tc.tile_pool(name="pc_tr", bufs=2, space="PSUM") as pct,
                tc.tile_pool(name="pc_w", bufs=3) as pcw,
            ):
                for t in t_order:
                    ch, sub = t // NSUB, t % NSUB
                    ps = pcp.tile([128, D], F32, tag="ps")
                    for kp in range(DK // 2):
                        nc.tensor.matmul(
                            out=ps,
                            lhsT=ctx_s[t // 2][:, 2 * kp : 2 * kp + 2,
                                               (t % 2) * 128 : (t % 2 + 1) * 128],
                            rhs=wo_sb[:, 2 * kp : 2 * kp + 2, :],
                            start=(kp == 0),
                            stop=False,
                            perf_mode=mybir.MatmulPerfMode.DoubleRow,
                        )
                    # residual: out-proj PSUM is 128x true (wo' = 2*Wo^T).
                    # fp8 part: transpose-inject xqT blocks against 128*I;
                    # error part: token-major fp8 xoerr against 128*I.
                    # (128 is the largest power of 2 in f8e4's range, max 240.)
                    for m in range(DK):
                        nc.tensor.matmul(
                            out=ps[:, m * 128 : (m + 1) * 128],
                            lhsT=xqT_sb[m // 2][:, m % 2, t * 128 : (t + 1) * 128],
                            rhs=ident128_f8,
                            start=False,
                            stop=False,
                            skip_group_check=True,
                        )
                    nc.tensor.matmul(
                        out=ps,
                        lhsT=ident128_f8,
                        rhs=xoerr_sb[:, t, :],
                        start=False,
                        stop=True,
                        skip_group_check=True,
                    )
                    layer_norm_step(ps, g1_b, bn1_b, xln1[ch][:, sub, :], pcw)
                    # PE-transpose LN1 output to feature-major, evacuating
                    # PSUM straight to fp8 (replaces per-tile DMA transposes
                    # + bf16 staging; ACT/DVE alternate to split the load).
                    tb = pct.tile([128, DK, 128], BF16, tag="tb")
                    for mm in range(DK):
                        nc.tensor.transpose(
                            out=tb[:, mm, :],
                            in_=xln1[ch][:, sub, mm * 128 : (mm + 1) * 128],
                            identity=ident_bf,
                        )
                    xdst = x1t[ch][:, :, sub * 128 : (sub + 1) * 128]
                    if t % 2 == 0:
                        nc.scalar.activation(out=xdst, in_=tb, func=AF.Identity)
                    else:
                        nc.vector.tensor_copy(xdst, tb)

            # ---------------- Phase D: FFN + LN2 + store ----------------------
            # hb is split into per-f-pair tiles so FFN2 trails FFN1 at
            # f-granularity; FFN2 runs in sub-pairs with double-buffered yps
            # so the next chunk isn't blocked on LN2 draining.
            with (
                tc.tile_pool(name="pf_h", bufs=4, space="PSUM") as pfh,
                tc.tile_pool(name="pf_y", bufs=2, space="PSUM") as pfy,
                tc.tile_pool(name="pf_hb", bufs=3) as pfhb,
                tc.tile_pool(name="pf_w", bufs=3) as pfw,
                tc.tile_pool(name="pf_o", bufs=3) as pfo,
            ):
                for ch in ch_order:
                    hb_p = [pfhb.tile([128, 2, WQ], F8, name=f"hb{i}", tag=f"hb{i}")
                            for i in range(FK // 2)]
                    for f in range(FK):
                        hp = pfh.tile([128, WQ], F32, tag="hp")
                        for kp in range(DK // 2):
                            nc.tensor.matmul(
                                out=hp,
                                lhsT=w1_sb[:, 2 * kp : 2 * kp + 2, f * 128 : (f + 1) * 128],
                                rhs=x1t[ch][:, 2 * kp : 2 * kp + 2, :],
                                start=(kp == 0),
                                stop=(kp == DK // 2 - 1),
                                perf_mode=mybir.MatmulPerfMode.DoubleRow,
                            )
                        # hb = 4x true = relu(psum * 2^-4).  The last chunk's
                        # relus run in the tail where DVE is free: split 50/50
                        # there; earlier chunks keep ACT-light 1-in-4 on DVE.
                        hdst = hb_p[f // 2][:, f % 2, :]
                        dve_relu = f % 4 == 3
                        if cfg.use_b1:
                            nc.scalar.activation(
                                out=hdst, in_=hp, func=AF.Relu,
                                bias=b1_sb[:, f : f + 1], scale=1.0 / 16.0,
                            )
                        elif dve_relu:
                            nc.vector.tensor_scalar(
                                out=hdst, in0=hp,
                                scalar1=1.0 / 16.0, scalar2=0.0,
                                op0=ALU.mult, op1=ALU.max,
                            )
                        else:
                            nc.scalar.activation(
                                out=hdst, in_=hp, func=AF.Relu,
                                scale=1.0 / 16.0,
                            )
                    for half in range(NSUB // 2):
                        yps = [pfy.tile([128, D], F32, name=f"y{i}", tag=f"y{i}")
                               for i in range(2)]
                        for fp in range(FK // 2):
                            for s2 in range(2):
                                c0 = (2 * half + s2) * 128
                                nc.tensor.matmul(
                                    out=yps[s2],
                                    lhsT=hb_p[fp][:, :, c0 : c0 + 128],
                                    rhs=w2_sb[:, 2 * fp : 2 * fp + 2, :],
                                    start=(fp == 0),
                                    stop=False,
                                    perf_mode=mybir.MatmulPerfMode.DoubleRow,
                                )
                        for s2 in range(2):
                            sub = 2 * half + s2
                            t = ch * NSUB + sub
                            # residual: += 256 * xln1 (FFN2 PSUM is 256x true)
                            nc.tensor.matmul(
                                out=yps[s2],
                                lhsT=ident256,
                                rhs=xln1[ch][:, sub, :],
                                start=False,
                                stop=True,
                                skip_group_check=True,
                            )
                            ob = pfo.tile([128, D], F16, tag="ob")
                            # final-chunk LN2s: run one of each pair on ACT so
                            # the two chains go in parallel in the drain tail
                            layer_norm_step(
                                yps[s2], g2_b, bn2_b, ob, pfw,
                                act_stats=(ch == ch_order[-1] and s2 == 1),
                            )
                            if cfg.use_b2:
                                nc.vector.tensor_add(ob, ob, b2_b)
                            nc.sync.dma_start(
                                out=out_d[t * 128 : (t + 1) * 128, :], in_=ob
                            )

            pw2_cm.__exit__(None, None, None)
            pd_cm.__exit__(None, None, None)
            pw1_cm.__exit__(None, None, None)
            pc_cm.__exit__(None, None, None)
            pctx_cm.__exit__(None, None, None)

        for _ in range(n_bodies):
            emit_body()

    nc.compile()
    return nc


# ---------------------------------------------------------------------------
# host side
# ---------------------------------------------------------------------------

def build_masks(cfg: Cfg, half: int) -> np.ndarray:
    """Additive masks [128, NSLOT, 2, 2*CHUNK] fp8: 0 keep / NEG drop.

    Layout of last dim: (jj 2) x CHUNK queries (head-independent); tile
    index dim2 selects k-tile pair T-2 or T-1 of the slot.
    """
    E = cfg.slot_E()
    chunks = cfg.slot_chunks(half)
    qs_l = cfg.slot_qs(half)
    m = np.zeros((128, cfg.NSLOT, 2, 2 * cfg.CHUNK), np.float32)
    k_loc = np.arange(128)[:, None]
    q_loc = np.arange(cfg.CHUNK)[None, :]
    for s, (ci, qs) in enumerate(zip(chunks, qs_l)):
        T = E[s] // 2
        for ti, tpair in enumerate((T - 2, T - 1)):
            for jj in range(2):
                j = 2 * tpair + jj
                keep = (qs + q_loc) >= (j * cfg.KT + k_loc)
                bias = np.where(keep, 0.0, NEG)
                m[:, s, ti, jj * cfg.CHUNK : (jj + 1) * cfg.CHUNK] = bias
    return m.astype(ml_dtypes.float8_e4m3)


def prepare_weights(inputs: dict, cfg: Cfg) -> dict:
    """fp8-scaled transposed weight tensors baked into the NEFF as constants."""
    Wqkv = np.asarray(inputs["Wqkv"], np.float32)
    Wo = np.asarray(inputs["Wo"], np.float32)
    W1 = np.asarray(inputs["W1"], np.float32)
    W2 = np.asarray(inputs["W2"], np.float32)
    D = cfg.D
    scale = 1.0 / np.sqrt(np.float32(cfg.HD))
    f8 = ml_dtypes.float8_e4m3
    wqkvT = np.concatenate(
        [
            np.ascontiguousarray(Wqkv[0:D].T) * (scale * 512.0),
            np.ascontiguousarray(Wqkv[D : 2 * D].T) * 64.0,
            np.ascontiguousarray(Wqkv[2 * D : 3 * D].T) * 64.0,
        ],
        axis=1,
    )
    return {
        # wo' at 2x (not 64x) so the out-proj PSUM sits at 128x true scale,
        # letting the residual enter via fp8 128*I identity matmuls.
        "wqkvT": wqkvT.astype(f8),
        "woT": (np.ascontiguousarray(Wo.T) * 2.0).astype(f8),
        "w1T": (np.ascontiguousarray(W1.T) * 64.0).astype(f8),
        "w2T": (np.ascontiguousarray(W2.T) * 64.0).astype(f8),
    }


def host_prepare(inputs: dict, cfg: Cfg):
    x = np.asarray(inputs["x"], np.float32)
    bqkv = np.asarray(inputs["bqkv"], np.float32)
    Wo = np.asarray(inputs["Wo"], np.float32)
    bo = np.asarray(inputs["bo"], np.float32)
    b1 = np.asarray(inputs["b1"], np.float32)
    b2 = np.asarray(inputs["b2"], np.float32)
    g1 = np.asarray(inputs["g1"], np.float32)
    bn1 = np.asarray(inputs["bn1"], np.float32)
    g2 = np.asarray(inputs["g2"], np.float32)
    bn2 = np.asarray(inputs["bn2"], np.float32)

    D = cfg.D
    scale = 1.0 / np.sqrt(np.float32(cfg.HD))
    f8 = ml_dtypes.float8_e4m3
    bq = bqkv[0:D] * scale
    bk = bqkv[D : 2 * D]
    bv = bqkv[2 * D : 3 * D]

    in_maps = []
    own_idx_per_core = []
    for c in range(2 * cfg.B):
        b = c // 2
        half = c % 2
        own_idx = np.concatenate(
            [np.arange(qs, qs + cfg.CHUNK) for qs in cfg.slot_qs(half)]
        )
        own_idx_per_core.append(own_idx)
        xb = x[b]
        xown = xb[own_idx] + bo[None, :]
        if cfg.use_bv:
            # bv flows through attention as a constant added to every v;
            # since attention rows sum to 1, it adds bv @ Wo^T to attn_out.
            xown = xown + (bv @ Wo.T)[None, :]
        xq8 = xb[own_idx].astype(f8)
        im = {
            "xT": np.ascontiguousarray(xb.T).astype(f8),
            "xoerr": (xown - xq8.astype(np.float32)).astype(f8),
        }
        if cfg.use_bq:
            im["bq"] = bq
        if cfg.use_bk:
            im["bk"] = bk
        if cfg.use_bv:
            im["bv"] = bv
        if cfg.use_b1:
            im["b1"] = b1 * 4.0
        if cfg.use_b2:
            im["b2"] = np.tile(b2[None, :], (128, 1))
        if cfg.use_g1:
            im["g1v"] = np.tile(g1[None, :], (128, 1))
        if cfg.use_bn1:
            im["bn1v"] = np.tile(bn1[None, :], (128, 1))
        if cfg.use_g2:
            im["g2v"] = np.tile(g2[None, :], (128, 1))
        if cfg.use_bn2:
            im["bn2v"] = np.tile(bn2[None, :], (128, 1))
        in_maps.append(im)
    return in_maps, own_idx_per_core


def make_cfg(inputs: dict) -> Cfg:
    x = np.asarray(inputs["x"])
    B, S, D = x.shape
    F = np.asarray(inputs["W1"]).shape[0]
    bqkv = np.asarray(inputs["bqkv"], np.float32)
    cfg = Cfg(
        B=B, S=S, D=D, F=F,
        use_bq=bool(np.any(bqkv[0:D])),
        use_bk=bool(np.any(bqkv[D : 2 * D])),
        use_bv=bool(np.any(bqkv[2 * D : 3 * D])),
        use_b1=bool(np.any(np.asarray(inputs["b1"]))),
        use_b2=bool(np.any(np.asarray(inputs["b2"]))),
        use_g1=not bool(np.all(np.asarray(inputs["g1"]) == 1.0)),
        use_bn1=bool(np.any(np.asarray(inputs["bn1"]))),
        use_g2=not bool(np.all(np.asarray(inputs["g2"]) == 1.0)),
        use_bn2=bool(np.any(np.asarray(inputs["bn2"]))),
    )
    return cfg


_NC_CACHE: dict = {}
_WEIGHTS: dict = {}

TRACE = False
LAST_RESULT = None


def kernel(**inputs) -> np.ndarray:
    global LAST_RESULT, _WEIGHTS
    cfg = make_cfg(inputs)
    w = prepare_weights(inputs, cfg)
    whash = hash(tuple(v.tobytes() for v in w.values()))
    key = (tuple(sorted(cfg.__dict__.items())), whash)
    if key not in _NC_CACHE:
        _WEIGHTS = w
        _NC_CACHE[key] = build_nc(cfg)
    nc = _NC_CACHE[key]

    in_maps, own_idx_per_core = host_prepare(inputs, cfg)
    ncores = 2 * cfg.B
    res = run_bass_kernel_spmd(
        nc, in_maps, core_ids=list(range(ncores)), trace=TRACE
    )
    LAST_RESULT = res

    out = np.empty((cfg.B, cfg.S, cfg.D), np.float32)
    for c in range(ncores):
        out[c // 2, own_idx_per_core[c]] = res.results[c]["out"].astype(np.float32)
    return out

